# revision 12
# baseline (speedup 1.0000x reference)
"""Trainium2 Bass kernel for nn_ADConv (adaptive-basis conv).

Math (per image, per pixel q=(h,w)):
  h1  = tanh(bn1(conv3x3(x)))                      # [64, H, W]
  bc  = tanh(bn2(conv3x3(h1)))                     # [96, H, W], channel = 6f+t
  PB[c,t,q]   = sum_k x[c, q+dk] * B[t,k]          # depthwise basis conv
  u[c,f,q]    = sum_t PB[c,t,q] * bc[6f+t, wq, hq] # per-pixel bilinear (DVE)
  out[o,w,h]  = sum_{c,f} coef[o, 16c+f] * u[c,f,q]

Sharding: data-parallel, batch 16 -> 2 images per NeuronCore, params
replicated. Everything computed in bf16 (fp32 PSUM accumulation).
"""

import os
import sys

import numpy as np

sys.path.insert(0, "/opt/trn_rl_repo")

import ml_dtypes

import concourse.bacc as bacc
import concourse.bass as bass
import concourse.mybir as mybir
import concourse.tile as tile
from concourse.ap import AP
from concourse.bass_utils import run_bass_kernel_spmd

BF16 = mybir.dt.bfloat16
F32 = mybir.dt.float32
AF = mybir.ActivationFunctionType
ALU = mybir.AluOpType

N_CORES = 8
IMGS = 2           # images per core
C = 64             # input channels
INTER = 64         # conv1 out channels
BCH = 96           # conv2 out channels = 16f * 6t
NT = 6             # TOTAL_BASES
NF = 16            # NUM_FA
O = 128            # output channels
H = W = 64
HP = 66            # padded spatial
Q = H * W          # 4096 pixels
RC = 8             # rows per chunk
NCHUNK = H // RC   # 8 chunks of 512 px
CH = RC * W        # 512 px per chunk
BN_EPS = 1e-5

_CACHE = {}


def _pbcast_src(tile_ap: AP, part_row: int, part_pitch: int, dims, offset_elems: int):
    """Manual AP: read from partition `part_row` of an SBUF tile, broadcast
    across 64 partitions (leading 0-stride dim), with free dims `dims`
    (list of [step, count]) starting at byte/elem offset `offset_elems`."""
    base = tile_ap  # AP covering whole tile, standard layout
    # Flat-element convention: partition p lives at p * part_pitch.
    off = part_row * part_pitch + offset_elems
    return AP(base.tensor, base.offset + off, [[0, 64]] + list(dims))


def build_graph():
    nc = bacc.Bacc(None, target_bir_lowering=False)

    xs = nc.declare_dram_parameter("xs", [IMGS, C, H, W], BF16, isOutput=False)
    w1t = nc.declare_dram_parameter("w1t", [C, 9, INTER], BF16, isOutput=False)
    s1 = nc.declare_dram_parameter("s1", [INTER, 1], F32, isOutput=False)
    b1 = nc.declare_dram_parameter("b1", [INTER, 1], F32, isOutput=False)
    w2t = nc.declare_dram_parameter("w2t", [INTER, 9, BCH], BF16, isOutput=False)
    s2 = nc.declare_dram_parameter("s2", [BCH, 1], F32, isOutput=False)
    b2 = nc.declare_dram_parameter("b2", [BCH, 1], F32, isOutput=False)
    wpb = nc.declare_dram_parameter("wpb", [C, 9, NT * C], BF16, isOutput=False)
    coefp = nc.declare_dram_parameter("coefp", [128, NF, O], BF16, isOutput=False)
    out = nc.declare_dram_parameter("out", [IMGS, O, W, H], F32, isOutput=True)

    with tile.TileContext(nc) as tc:
        with (
            tc.tile_pool(name="persist", bufs=1) as pp,
            tc.tile_pool(name="bcrep", bufs=3) as bp,
            tc.tile_pool(name="uu", bufs=2) as up,
            tc.tile_pool(name="tmp", bufs=2) as tp,
            tc.tile_pool(name="osb", bufs=2) as op_,
            tc.tile_pool(name="dramb", bufs=1, space=bass.MemorySpace.DRAM) as dp,
            tc.tile_pool(name="ps_conv", bufs=1, space=bass.MemorySpace.PSUM) as pcv,
            tc.tile_pool(name="ps_pb", bufs=1, space=bass.MemorySpace.PSUM) as ppb,
            tc.tile_pool(name="ps_fin", bufs=1, space=bass.MemorySpace.PSUM) as pfn,
        ):
            # ---- persistent SBUF ----
            w1sb = pp.tile([C, 9, INTER], BF16, tag="w1sb")
            w2sb = pp.tile([INTER, 9, BCH], BF16, tag="w2sb")
            wpbsb = pp.tile([C, 9, NT * C], BF16, tag="wpbsb")
            coefsb = pp.tile([128, NF, O], BF16, tag="coefsb")
            s1sb = pp.tile([INTER, 1], F32, tag="s1sb")
            b1sb = pp.tile([INTER, 1], F32, tag="b1sb")
            s2sb = pp.tile([BCH, 1], F32, tag="s2sb")
            b2sb = pp.tile([BCH, 1], F32, tag="b2sb")
            nc.sync.dma_start(w1sb[:], w1t[:])
            nc.sync.dma_start(w2sb[:], w2t[:])
            nc.sync.dma_start(wpbsb[:], wpb[:])
            nc.sync.dma_start(coefsb[:], coefp[:])
            nc.sync.dma_start(s1sb[:], s1[:])
            nc.sync.dma_start(b1sb[:], b1[:])
            nc.sync.dma_start(s2sb[:], s2[:])
            nc.sync.dma_start(b2sb[:], b2[:])

            xpad = []
            hpad = []
            bcs = []
            for i in range(IMGS):
                xp = pp.tile([C, HP, HP], BF16, tag=f"xpad{i}", name=f"xpad{i}")
                hp = pp.tile([INTER, HP, HP], BF16, tag=f"hpad{i}", name=f"hpad{i}")
                bi = pp.tile([BCH, H, W], BF16, tag=f"bc{i}", name=f"bc{i}")
                nc.vector.memset(xp[:], 0.0)
                nc.vector.memset(hp[:], 0.0)
                nc.sync.dma_start(xp[:, 1 : H + 1, 1 : W + 1], xs[i])
                xpad.append(xp)
                hpad.append(hp)
                bcs.append(bi)
            pbt = [pp.tile([128, Q], BF16, tag=f"pbt{t}", name=f"pbt{t}") for t in range(NT)]
            bcd = [
                dp.tile([BCH, H, W], BF16, tag=f"bcd{i}", name=f"bcd{i}")
                for i in range(IMGS)
            ]

            # ---- stage 1+2+PB per image ----
            for i in range(IMGS):
                # conv1 -> bn -> tanh -> hpad
                for g in range(2):  # chunk groups of 4 (PSUM budget)
                    cps = pcv.tile([INTER, 4, RC, W], F32, tag="convps")
                    for k in range(9):
                        ki, kj = k // 3, k % 3
                        lhsT = w1sb[:, k, :]
                        for j in range(4):
                            h0 = (g * 4 + j) * RC
                            rhs = xpad[i][:, h0 + ki : h0 + ki + RC, kj : kj + W]
                            nc.tensor.matmul(
                                cps[:, j], lhsT, rhs, start=(k == 0), stop=(k == 8)
                            )
                    for j in range(4):
                        h0 = (g * 4 + j) * RC
                        nc.scalar.activation(
                            hpad[i][:, h0 + 1 : h0 + 1 + RC, 1 : W + 1],
                            cps[:, j],
                            AF.Tanh,
                            bias=b1sb[:],
                            scale=s1sb[:],
                        )
                # conv2 -> bn -> tanh -> bc
                for g in range(2):
                    cps = pcv.tile([BCH, 4, RC, W], F32, tag="convps")
                    for k in range(9):
                        ki, kj = k // 3, k % 3
                        lhsT = w2sb[:, k, :]
                        for j in range(4):
                            h0 = (g * 4 + j) * RC
                            rhs = hpad[i][:, h0 + ki : h0 + ki + RC, kj : kj + W]
                            nc.tensor.matmul(
                                cps[:, j], lhsT, rhs, start=(k == 0), stop=(k == 8)
                            )
                    for j in range(4):
                        h0 = (g * 4 + j) * RC
                        # transposed store: bcT[ch, w_out, h_bc] so later
                        # per-pixel reads are contiguous
                        nc.scalar.activation(
                            bcs[i][:, :, h0 : h0 + RC].transpose([0, 2, 1]),
                            cps[:, j],
                            AF.Tanh,
                            bias=b2sb[:],
                            scale=s2sb[:],
                        )
            for i in range(IMGS):
                nc.sync.dma_start(bcd[i][:], bcs[i][:])

            # ---- per chunk: PB (PE) -> per-pixel bilinear (DVE+Pool) -> final ----
            bc_pitch = H * W
            POOL_F = (11, 12, 13, 14, 15)  # f values computed on GpSimd
            for ch in range(NCHUNK):
                h0 = ch * RC
                c0 = h0 * W
                for i in range(IMGS):
                    pps = ppb.tile([128, 3, RC, W], F32, tag="pbps")
                    for j3 in range(3):
                        for k in range(9):
                            ki, kj = k // 3, k % 3
                            lhsT = wpbsb[:, k, j3 * 128 : (j3 + 1) * 128]
                            rhs = xpad[i][:, h0 + ki : h0 + ki + RC, kj : kj + W]
                            nc.tensor.matmul(
                                pps[:, j3], lhsT, rhs, start=(k == 0), stop=(k == 8)
                            )
                    for j3 in range(3):
                        nc.scalar.copy(
                            pbt[2 * j3][i * 64 : (i + 1) * 64, c0 : c0 + CH],
                            pps[0:64, j3].opt(),
                        )
                        nc.scalar.copy(
                            pbt[2 * j3 + 1][i * 64 : (i + 1) * 64, c0 : c0 + CH],
                            pps[64:128, j3].opt(),
                        )
                uu = up.tile([128, NF * CH], BF16, tag="uu")
                for f in range(NF):
                    eng = nc.gpsimd if f in POOL_F else nc.vector
                    bcr = bp.tile([128, NT * CH], BF16, tag="bcr")
                    bcr_full = bcr[:]
                    for i in range(IMGS):
                        dfull = bcd[i][:]
                        srcap = AP(
                            dfull.tensor,
                            dfull.offset + f * NT * bc_pitch + c0,
                            [[0, 64], [bc_pitch, NT], [1, CH]],
                        )
                        dstap = AP(
                            bcr_full.tensor,
                            bcr_full.offset + i * 64 * (NT * CH),
                            [[NT * CH, 64], [CH, NT], [1, CH]],
                        )
                        nc.sync.dma_start(dstap, srcap)
                    uslice = uu[:, f * CH : (f + 1) * CH]
                    use_pool = f in POOL_F
                    for t in range(NT):
                        pbin = pbt[t][:, c0 : c0 + CH]
                        bslice = bcr[:, t * CH : (t + 1) * CH]
                        if t == 0:
                            if use_pool:
                                eng.tensor_mul(uslice, pbin, bslice)
                            else:
                                eng.scalar_tensor_tensor(
                                    uslice, pbin, 1.0, bslice, ALU.mult, ALU.mult
                                )
                        else:
                            tmp = tp.tile([128, CH], BF16, tag="tmp")
                            if use_pool:
                                eng.tensor_mul(tmp[:], pbin, bslice)
                                eng.tensor_add(uslice, tmp[:], uslice)
                            else:
                                eng.scalar_tensor_tensor(
                                    tmp[:], pbin, 1.0, bslice, ALU.mult, ALU.mult
                                )
                                eng.scalar_tensor_tensor(
                                    uslice, tmp[:], 1.0, uslice, ALU.mult, ALU.add
                                )
                # final 1x1: out[o, q] = sum_{c,f} coefp * u
                for i in range(IMGS):
                    fps = pfn.tile([O, RC, W], F32, tag="finps")
                    for f in range(NF):
                        nc.tensor.matmul(
                            fps[:],
                            coefsb[i * 64 : (i + 1) * 64, f, :],
                            uu[i * 64 : (i + 1) * 64, f * CH : (f + 1) * CH],
                            start=(f == 0),
                            stop=(f == NF - 1),
                        )
                    osb = op_.tile([O, W, RC], F32, tag="osb")
                    # store transposed: osb[o, w, h]
                    nc.scalar.copy(osb[:].transpose([0, 2, 1]), fps[:])
                    nc.sync.dma_start(out[i, :, :, h0 : h0 + RC], osb[:])

    nc.compile()
    return nc


def _prep_params(inputs):
    bf16 = ml_dtypes.bfloat16
    f32 = np.float32
    c1w = np.asarray(inputs["conv1_w"], f32)
    c2w = np.asarray(inputs["conv2_w"], f32)
    bases = np.asarray(inputs["bases"], f32)
    coef = np.asarray(inputs["coef"], f32)

    s1 = np.asarray(inputs["bn1_gamma"], f32) / np.sqrt(
        np.asarray(inputs["bn1_var"], f32) + BN_EPS
    )
    b1 = (np.asarray(inputs["conv1_b"], f32) - np.asarray(inputs["bn1_mean"], f32)) * s1 + np.asarray(
        inputs["bn1_beta"], f32
    )
    s2 = np.asarray(inputs["bn2_gamma"], f32) / np.sqrt(
        np.asarray(inputs["bn2_var"], f32) + BN_EPS
    )
    b2 = (np.asarray(inputs["conv2_b"], f32) - np.asarray(inputs["bn2_mean"], f32)) * s2 + np.asarray(
        inputs["bn2_beta"], f32
    )

    w1t = np.ascontiguousarray(c1w.transpose(1, 2, 3, 0).reshape(C, 9, INTER))
    w2t = np.ascontiguousarray(c2w.transpose(1, 2, 3, 0).reshape(INTER, 9, BCH))

    wpb = np.zeros((C, 9, NT * C), f32)
    for t in range(NT):
        for c in range(C):
            wpb[c, :, t * C + c] = bases[t, :]

    cview = coef.reshape(O, C, NF)  # coef[o, 16c+f]
    coefp64 = np.ascontiguousarray(cview.transpose(1, 2, 0))  # [c, f, o]
    coefp = np.concatenate([coefp64, coefp64], axis=0)  # [128, f, o]

    return {
        "w1t": w1t.astype(bf16),
        "s1": s1.reshape(-1, 1).astype(f32),
        "b1": b1.reshape(-1, 1).astype(f32),
        "w2t": w2t.astype(bf16),
        "s2": s2.reshape(-1, 1).astype(f32),
        "b2": b2.reshape(-1, 1).astype(f32),
        "wpb": wpb.astype(bf16),
        "coefp": coefp.astype(bf16),
    }


def kernel(**inputs):
    if "nc" not in _CACHE:
        _CACHE["nc"] = build_graph()
    nc = _CACHE["nc"]

    params = _prep_params(inputs)
    x = np.asarray(inputs["x"], np.float32).astype(ml_dtypes.bfloat16)

    in_maps = []
    for core in range(N_CORES):
        m = dict(params)
        m["xs"] = np.ascontiguousarray(x[core * IMGS : (core + 1) * IMGS])
        in_maps.append(m)

    res = run_bass_kernel_spmd(nc, in_maps, core_ids=list(range(N_CORES)))
    outs = [r["out"] for r in res.results]
    return np.concatenate(outs, axis=0).astype(np.float32)


def _install_ntff_hook():
    """Shim antenv.axon_hooks with the trn_boot ctypes NTFF hook."""
    import types

    try:
        from antenv.axon_hooks import get_axon_ntff_profile_hook  # noqa
        return
    except ImportError:
        pass
    sys.path.insert(0, "/root/.axon_site/trn_agent_boot")
    import trn_boot

    hook = trn_boot._ntff_profile_via_ctypes("/opt/axon/libaxon_pjrt.so")
    mod_pkg = sys.modules.get("antenv")
    if mod_pkg is None:
        mod_pkg = types.ModuleType("antenv")
        sys.modules["antenv"] = mod_pkg
    mod = types.ModuleType("antenv.axon_hooks")
    mod.get_axon_ntff_profile_hook = lambda: hook
    mod.set_axon_ntff_profile_hook = lambda h: None
    sys.modules["antenv.axon_hooks"] = mod
    mod_pkg.axon_hooks = mod


def run_timed(inputs):
    """Run once with NTFF tracing; return exec_time_ns (or None)."""
    _install_ntff_hook()
    if "nc" not in _CACHE:
        _CACHE["nc"] = build_graph()
    nc = _CACHE["nc"]
    params = _prep_params(inputs)
    x = np.asarray(inputs["x"], np.float32).astype(ml_dtypes.bfloat16)
    in_maps = []
    for core in range(N_CORES):
        m = dict(params)
        m["xs"] = np.ascontiguousarray(x[core * IMGS : (core + 1) * IMGS])
        in_maps.append(m)
    res = run_bass_kernel_spmd(
        nc, in_maps, core_ids=list(range(N_CORES)), trace=True
    )
    print("trace profile_json:", res.profile_json)
    return res.exec_time_ns


if __name__ == "__main__":
    rng = np.random.default_rng(0)
    fake = {
        "x": rng.standard_normal((16, 64, 64, 64), np.float32),
        "conv1_w": rng.standard_normal((64, 64, 3, 3), np.float32) * 0.05,
        "conv1_b": rng.standard_normal((64,), np.float32) * 0.05,
        "bn1_gamma": rng.uniform(0.5, 1.5, (64,)).astype(np.float32),
        "bn1_beta": rng.standard_normal((64,), np.float32) * 0.05,
        "bn1_mean": rng.standard_normal((64,), np.float32) * 0.05,
        "bn1_var": rng.uniform(0.5, 1.5, (64,)).astype(np.float32),
        "conv2_w": rng.standard_normal((96, 64, 3, 3), np.float32) * 0.05,
        "conv2_b": rng.standard_normal((96,), np.float32) * 0.05,
        "bn2_gamma": rng.uniform(0.5, 1.5, (96,)).astype(np.float32),
        "bn2_beta": rng.standard_normal((96,), np.float32) * 0.05,
        "bn2_mean": rng.standard_normal((96,), np.float32) * 0.05,
        "bn2_var": rng.uniform(0.5, 1.5, (96,)).astype(np.float32),
        "bases": rng.standard_normal((6, 9), np.float32),
        "coef": rng.standard_normal((128, 1024), np.float32) * 0.02,
    }
    o = kernel(**fake)
    print("out", o.shape, o.dtype)


# revision 14
# speedup vs baseline: 1.0939x; 1.0939x over previous
"""Trainium2 Bass kernel for nn_ADConv (adaptive-basis conv).

Math (per image, per pixel q=(h,w)):
  h1  = tanh(bn1(conv3x3(x)))                      # [64, H, W]
  bc  = tanh(bn2(conv3x3(h1)))                     # [96, H, W], channel = 6f+t
  PB[c,t,q]   = sum_k x[c, q+dk] * B[t,k]          # depthwise basis conv
  u[c,f,q]    = sum_t PB[c,t,q] * bc[6f+t, wq, hq] # per-pixel bilinear (DVE)
  out[o,w,h]  = sum_{c,f} coef[o, 16c+f] * u[c,f,q]

Sharding: data-parallel, batch 16 -> 2 images per NeuronCore, params
replicated. Everything computed in bf16 (fp32 PSUM accumulation).
"""

import os
import sys

import numpy as np

sys.path.insert(0, "/opt/trn_rl_repo")

import ml_dtypes

import concourse.bacc as bacc
import concourse.bass as bass
import concourse.mybir as mybir
import concourse.tile as tile
from concourse.ap import AP
from concourse.bass_utils import run_bass_kernel_spmd

BF16 = mybir.dt.bfloat16
F32 = mybir.dt.float32
AF = mybir.ActivationFunctionType
ALU = mybir.AluOpType

N_CORES = 8
IMGS = 2           # images per core
C = 64             # input channels
INTER = 64         # conv1 out channels
BCH = 96           # conv2 out channels = 16f * 6t
NT = 6             # TOTAL_BASES
NF = 16            # NUM_FA
O = 128            # output channels
H = W = 64
HP = 66            # padded spatial
Q = H * W          # 4096 pixels
RC = 8             # rows per chunk
NCHUNK = H // RC   # 8 chunks of 512 px
CH = RC * W        # 512 px per chunk
BN_EPS = 1e-5

_CACHE = {}


def _pbcast_src(tile_ap: AP, part_row: int, part_pitch: int, dims, offset_elems: int):
    """Manual AP: read from partition `part_row` of an SBUF tile, broadcast
    across 64 partitions (leading 0-stride dim), with free dims `dims`
    (list of [step, count]) starting at byte/elem offset `offset_elems`."""
    base = tile_ap  # AP covering whole tile, standard layout
    # Flat-element convention: partition p lives at p * part_pitch.
    off = part_row * part_pitch + offset_elems
    return AP(base.tensor, base.offset + off, [[0, 64]] + list(dims))


def build_graph():
    nc = bacc.Bacc(None, target_bir_lowering=False)

    xs = nc.declare_dram_parameter("xs", [IMGS, C, H, W], BF16, isOutput=False)
    w1t = nc.declare_dram_parameter("w1t", [C, 9, INTER], BF16, isOutput=False)
    s1 = nc.declare_dram_parameter("s1", [INTER, 1], F32, isOutput=False)
    b1 = nc.declare_dram_parameter("b1", [INTER, 1], F32, isOutput=False)
    w2t = nc.declare_dram_parameter("w2t", [INTER, 9, BCH], BF16, isOutput=False)
    s2 = nc.declare_dram_parameter("s2", [BCH, 1], F32, isOutput=False)
    b2 = nc.declare_dram_parameter("b2", [BCH, 1], F32, isOutput=False)
    wpb = nc.declare_dram_parameter("wpb", [C, 9, NT * C], BF16, isOutput=False)
    coefp = nc.declare_dram_parameter("coefp", [128, NF, O], BF16, isOutput=False)
    out = nc.declare_dram_parameter("out", [IMGS, O, W, H], F32, isOutput=True)

    with tile.TileContext(nc) as tc:
        with (
            tc.tile_pool(name="persist", bufs=1) as pp,
            tc.tile_pool(name="bcrep", bufs=3) as bp,
            tc.tile_pool(name="uu", bufs=2) as up,
            tc.tile_pool(name="tmp", bufs=2) as tp,
            tc.tile_pool(name="osb", bufs=2) as op_,
            tc.tile_pool(name="dramb", bufs=1, space=bass.MemorySpace.DRAM) as dp,
            tc.tile_pool(name="ps_conv", bufs=1, space=bass.MemorySpace.PSUM) as pcv,
            tc.tile_pool(name="ps_pb", bufs=1, space=bass.MemorySpace.PSUM) as ppb,
            tc.tile_pool(name="ps_fin", bufs=1, space=bass.MemorySpace.PSUM) as pfn,
        ):
            # ---- persistent SBUF ----
            w1sb = pp.tile([C, 9, INTER], BF16, tag="w1sb")
            w2sb = pp.tile([INTER, 9, BCH], BF16, tag="w2sb")
            wpbsb = pp.tile([C, 9, NT * C], BF16, tag="wpbsb")
            coefsb = pp.tile([128, NF, O], BF16, tag="coefsb")
            s1sb = pp.tile([INTER, 1], F32, tag="s1sb")
            b1sb = pp.tile([INTER, 1], F32, tag="b1sb")
            s2sb = pp.tile([BCH, 1], F32, tag="s2sb")
            b2sb = pp.tile([BCH, 1], F32, tag="b2sb")
            nc.sync.dma_start(w1sb[:], w1t[:])
            nc.sync.dma_start(w2sb[:], w2t[:])
            nc.sync.dma_start(wpbsb[:], wpb[:])
            nc.sync.dma_start(coefsb[:], coefp[:])
            nc.sync.dma_start(s1sb[:], s1[:])
            nc.sync.dma_start(b1sb[:], b1[:])
            nc.sync.dma_start(s2sb[:], s2[:])
            nc.sync.dma_start(b2sb[:], b2[:])

            xpad = []
            hpad = []
            bcs = []
            for i in range(IMGS):
                xp = pp.tile([C, HP, HP], BF16, tag=f"xpad{i}", name=f"xpad{i}")
                hp = pp.tile([INTER, HP, HP], BF16, tag=f"hpad{i}", name=f"hpad{i}")
                bi = pp.tile([BCH, H, W], BF16, tag=f"bc{i}", name=f"bc{i}")
                nc.vector.memset(xp[:], 0.0)
                nc.vector.memset(hp[:], 0.0)
                nc.sync.dma_start(xp[:, 1 : H + 1, 1 : W + 1], xs[i])
                xpad.append(xp)
                hpad.append(hp)
                bcs.append(bi)
            pbt = [pp.tile([128, Q], BF16, tag=f"pbt{t}", name=f"pbt{t}") for t in range(NT)]
            bcd = [
                dp.tile([BCH, H, W], BF16, tag=f"bcd{i}", name=f"bcd{i}")
                for i in range(IMGS)
            ]

            # ---- stage 1+2+PB per image ----
            for i in range(IMGS):
                # conv1 -> bn -> tanh -> hpad
                for g in range(2):  # chunk groups of 4 (PSUM budget)
                    cps = pcv.tile([INTER, 4, RC, W], F32, tag="convps")
                    for k in range(9):
                        ki, kj = k // 3, k % 3
                        lhsT = w1sb[:, k, :]
                        for j in range(4):
                            h0 = (g * 4 + j) * RC
                            rhs = xpad[i][:, h0 + ki : h0 + ki + RC, kj : kj + W]
                            nc.tensor.matmul(
                                cps[:, j], lhsT, rhs, start=(k == 0), stop=(k == 8)
                            )
                    for j in range(4):
                        h0 = (g * 4 + j) * RC
                        nc.scalar.activation(
                            hpad[i][:, h0 + 1 : h0 + 1 + RC, 1 : W + 1],
                            cps[:, j],
                            AF.Tanh,
                            bias=b1sb[:],
                            scale=s1sb[:],
                        )
                # conv2 -> bn -> tanh -> bc
                for g in range(2):
                    cps = pcv.tile([BCH, 4, RC, W], F32, tag="convps")
                    for k in range(9):
                        ki, kj = k // 3, k % 3
                        lhsT = w2sb[:, k, :]
                        for j in range(4):
                            h0 = (g * 4 + j) * RC
                            rhs = hpad[i][:, h0 + ki : h0 + ki + RC, kj : kj + W]
                            nc.tensor.matmul(
                                cps[:, j], lhsT, rhs, start=(k == 0), stop=(k == 8)
                            )
                    for j in range(4):
                        h0 = (g * 4 + j) * RC
                        # transposed store: bcT[ch, w_out, h_bc] so later
                        # per-pixel reads are contiguous
                        nc.scalar.activation(
                            bcs[i][:, :, h0 : h0 + RC].transpose([0, 2, 1]),
                            cps[:, j],
                            AF.Tanh,
                            bias=b2sb[:],
                            scale=s2sb[:],
                        )
            for i in range(IMGS):
                nc.sync.dma_start(bcd[i][:], bcs[i][:])

            # ---- per chunk: PB (PE) -> per-pixel bilinear (DVE+Pool) -> final ----
            bc_pitch = H * W
            POOL_F = (11, 12, 13, 14, 15)  # f values computed on GpSimd
            for ch in range(NCHUNK):
                h0 = ch * RC
                c0 = h0 * W
                for i in range(IMGS):
                    pps = ppb.tile([128, 3, RC, W], F32, tag="pbps")
                    for j3 in range(3):
                        for k in range(9):
                            ki, kj = k // 3, k % 3
                            lhsT = wpbsb[:, k, j3 * 128 : (j3 + 1) * 128]
                            rhs = xpad[i][:, h0 + ki : h0 + ki + RC, kj : kj + W]
                            nc.tensor.matmul(
                                pps[:, j3], lhsT, rhs, start=(k == 0), stop=(k == 8)
                            )
                    for j3 in range(3):
                        nc.scalar.copy(
                            pbt[2 * j3][i * 64 : (i + 1) * 64, c0 : c0 + CH],
                            pps[0:64, j3].opt(),
                        )
                        nc.scalar.copy(
                            pbt[2 * j3 + 1][i * 64 : (i + 1) * 64, c0 : c0 + CH],
                            pps[64:128, j3].opt(),
                        )
                uu_v = up.tile([128, 11 * CH], BF16, tag="uuv")
                uu_p = up.tile([128, 5 * CH], BF16, tag="uup")
                for f in range(NF):
                    eng = nc.gpsimd if f in POOL_F else nc.vector
                    bcr = bp.tile([128, NT * CH], BF16, tag="bcr")
                    bcr_full = bcr[:]
                    for i in range(IMGS):
                        dfull = bcd[i][:]
                        srcap = AP(
                            dfull.tensor,
                            dfull.offset + f * NT * bc_pitch + c0,
                            [[0, 64], [bc_pitch, NT], [1, CH]],
                        )
                        dstap = AP(
                            bcr_full.tensor,
                            bcr_full.offset + i * 64 * (NT * CH),
                            [[NT * CH, 64], [CH, NT], [1, CH]],
                        )
                        nc.sync.dma_start(dstap, srcap)
                    use_pool = f in POOL_F
                    if use_pool:
                        fi = POOL_F.index(f)
                        uslice = uu_p[:, fi * CH : (fi + 1) * CH]
                    else:
                        uslice = uu_v[:, (f - sum(1 for p in POOL_F if p < f)) * CH :][:, :CH]
                    for t in range(NT):
                        pbin = pbt[t][:, c0 : c0 + CH]
                        bslice = bcr[:, t * CH : (t + 1) * CH]
                        if t == 0:
                            eng.tensor_mul(uslice, pbin, bslice)
                        else:
                            tag = "tmpp" if use_pool else "tmpv"
                            tmp = tp.tile([128, CH], BF16, tag=tag, name=tag)
                            eng.tensor_mul(tmp[:], pbin, bslice)
                            eng.tensor_add(uslice, tmp[:], uslice)
                # final 1x1: out[o, q] = sum_{c,f} coefp * u
                for i in range(IMGS):
                    fps = pfn.tile([O, RC, W], F32, tag="finps")
                    for f in range(NF):
                        if f in POOL_F:
                            fi = POOL_F.index(f)
                            usl = uu_p[i * 64 : (i + 1) * 64, fi * CH : (fi + 1) * CH]
                        else:
                            fv = f - sum(1 for p in POOL_F if p < f)
                            usl = uu_v[i * 64 : (i + 1) * 64, fv * CH : (fv + 1) * CH]
                        nc.tensor.matmul(
                            fps[:],
                            coefsb[i * 64 : (i + 1) * 64, f, :],
                            usl,
                            start=(f == 0),
                            stop=(f == NF - 1),
                        )
                    osb = op_.tile([O, W, RC], F32, tag="osb")
                    # store transposed: osb[o, w, h]
                    nc.scalar.copy(osb[:].transpose([0, 2, 1]), fps[:])
                    nc.sync.dma_start(out[i, :, :, h0 : h0 + RC], osb[:])

    nc.compile()
    return nc


def _prep_params(inputs):
    bf16 = ml_dtypes.bfloat16
    f32 = np.float32
    c1w = np.asarray(inputs["conv1_w"], f32)
    c2w = np.asarray(inputs["conv2_w"], f32)
    bases = np.asarray(inputs["bases"], f32)
    coef = np.asarray(inputs["coef"], f32)

    s1 = np.asarray(inputs["bn1_gamma"], f32) / np.sqrt(
        np.asarray(inputs["bn1_var"], f32) + BN_EPS
    )
    b1 = (np.asarray(inputs["conv1_b"], f32) - np.asarray(inputs["bn1_mean"], f32)) * s1 + np.asarray(
        inputs["bn1_beta"], f32
    )
    s2 = np.asarray(inputs["bn2_gamma"], f32) / np.sqrt(
        np.asarray(inputs["bn2_var"], f32) + BN_EPS
    )
    b2 = (np.asarray(inputs["conv2_b"], f32) - np.asarray(inputs["bn2_mean"], f32)) * s2 + np.asarray(
        inputs["bn2_beta"], f32
    )

    w1t = np.ascontiguousarray(c1w.transpose(1, 2, 3, 0).reshape(C, 9, INTER))
    w2t = np.ascontiguousarray(c2w.transpose(1, 2, 3, 0).reshape(INTER, 9, BCH))

    wpb = np.zeros((C, 9, NT * C), f32)
    for t in range(NT):
        for c in range(C):
            wpb[c, :, t * C + c] = bases[t, :]

    cview = coef.reshape(O, C, NF)  # coef[o, 16c+f]
    coefp64 = np.ascontiguousarray(cview.transpose(1, 2, 0))  # [c, f, o]
    coefp = np.concatenate([coefp64, coefp64], axis=0)  # [128, f, o]

    return {
        "w1t": w1t.astype(bf16),
        "s1": s1.reshape(-1, 1).astype(f32),
        "b1": b1.reshape(-1, 1).astype(f32),
        "w2t": w2t.astype(bf16),
        "s2": s2.reshape(-1, 1).astype(f32),
        "b2": b2.reshape(-1, 1).astype(f32),
        "wpb": wpb.astype(bf16),
        "coefp": coefp.astype(bf16),
    }


def kernel(**inputs):
    if "nc" not in _CACHE:
        _CACHE["nc"] = build_graph()
    nc = _CACHE["nc"]

    params = _prep_params(inputs)
    x = np.asarray(inputs["x"], np.float32).astype(ml_dtypes.bfloat16)

    in_maps = []
    for core in range(N_CORES):
        m = dict(params)
        m["xs"] = np.ascontiguousarray(x[core * IMGS : (core + 1) * IMGS])
        in_maps.append(m)

    res = run_bass_kernel_spmd(nc, in_maps, core_ids=list(range(N_CORES)))
    outs = [r["out"] for r in res.results]
    return np.concatenate(outs, axis=0).astype(np.float32)


def _install_ntff_hook():
    """Shim antenv.axon_hooks with the trn_boot ctypes NTFF hook."""
    import types

    try:
        from antenv.axon_hooks import get_axon_ntff_profile_hook  # noqa
        return
    except ImportError:
        pass
    sys.path.insert(0, "/root/.axon_site/trn_agent_boot")
    import trn_boot

    hook = trn_boot._ntff_profile_via_ctypes("/opt/axon/libaxon_pjrt.so")
    mod_pkg = sys.modules.get("antenv")
    if mod_pkg is None:
        mod_pkg = types.ModuleType("antenv")
        sys.modules["antenv"] = mod_pkg
    mod = types.ModuleType("antenv.axon_hooks")
    mod.get_axon_ntff_profile_hook = lambda: hook
    mod.set_axon_ntff_profile_hook = lambda h: None
    sys.modules["antenv.axon_hooks"] = mod
    mod_pkg.axon_hooks = mod


def run_timed(inputs):
    """Run once with NTFF tracing; return exec_time_ns (or None)."""
    _install_ntff_hook()
    if "nc" not in _CACHE:
        _CACHE["nc"] = build_graph()
    nc = _CACHE["nc"]
    params = _prep_params(inputs)
    x = np.asarray(inputs["x"], np.float32).astype(ml_dtypes.bfloat16)
    in_maps = []
    for core in range(N_CORES):
        m = dict(params)
        m["xs"] = np.ascontiguousarray(x[core * IMGS : (core + 1) * IMGS])
        in_maps.append(m)
    res = run_bass_kernel_spmd(
        nc, in_maps, core_ids=list(range(N_CORES)), trace=True
    )
    print("trace profile_json:", res.profile_json)
    _CACHE["last_res"] = res
    return res.exec_time_ns


if __name__ == "__main__":
    rng = np.random.default_rng(0)
    fake = {
        "x": rng.standard_normal((16, 64, 64, 64), np.float32),
        "conv1_w": rng.standard_normal((64, 64, 3, 3), np.float32) * 0.05,
        "conv1_b": rng.standard_normal((64,), np.float32) * 0.05,
        "bn1_gamma": rng.uniform(0.5, 1.5, (64,)).astype(np.float32),
        "bn1_beta": rng.standard_normal((64,), np.float32) * 0.05,
        "bn1_mean": rng.standard_normal((64,), np.float32) * 0.05,
        "bn1_var": rng.uniform(0.5, 1.5, (64,)).astype(np.float32),
        "conv2_w": rng.standard_normal((96, 64, 3, 3), np.float32) * 0.05,
        "conv2_b": rng.standard_normal((96,), np.float32) * 0.05,
        "bn2_gamma": rng.uniform(0.5, 1.5, (96,)).astype(np.float32),
        "bn2_beta": rng.standard_normal((96,), np.float32) * 0.05,
        "bn2_mean": rng.standard_normal((96,), np.float32) * 0.05,
        "bn2_var": rng.uniform(0.5, 1.5, (96,)).astype(np.float32),
        "bases": rng.standard_normal((6, 9), np.float32),
        "coef": rng.standard_normal((128, 1024), np.float32) * 0.02,
    }
    o = kernel(**fake)
    print("out", o.shape, o.dtype)


# revision 16
# speedup vs baseline: 1.4345x; 1.3113x over previous
"""Trainium2 Bass kernel for nn_ADConv (adaptive-basis conv).

Math (per image, per pixel q=(h,w)):
  h1  = tanh(bn1(conv3x3(x)))                      # [64, H, W]
  bc  = tanh(bn2(conv3x3(h1)))                     # [96, H, W], channel = 6f+t
  PB[c,t,q]   = sum_k x[c, q+dk] * B[t,k]          # depthwise basis conv
  u[c,f,q]    = sum_t PB[c,t,q] * bc[6f+t, wq, hq] # per-pixel bilinear (DVE)
  out[o,w,h]  = sum_{c,f} coef[o, 16c+f] * u[c,f,q]

Sharding: data-parallel, batch 16 -> 2 images per NeuronCore, params
replicated. Everything computed in bf16 (fp32 PSUM accumulation).
"""

import os
import sys

import numpy as np

sys.path.insert(0, "/opt/trn_rl_repo")

import ml_dtypes

import concourse.bacc as bacc
import concourse.bass as bass
import concourse.mybir as mybir
import concourse.tile as tile
from concourse.ap import AP
from concourse.bass_utils import run_bass_kernel_spmd

BF16 = mybir.dt.bfloat16
F32 = mybir.dt.float32
AF = mybir.ActivationFunctionType
ALU = mybir.AluOpType

N_CORES = 8
IMGS = 2           # images per core
C = 64             # input channels
INTER = 64         # conv1 out channels
BCH = 96           # conv2 out channels = 16f * 6t
NT = 6             # TOTAL_BASES
NF = 16            # NUM_FA
O = 128            # output channels
H = W = 64
HP = 66            # padded spatial
Q = H * W          # 4096 pixels
RC = 8             # rows per chunk
NCHUNK = H // RC   # 8 chunks of 512 px
CH = RC * W        # 512 px per chunk
BN_EPS = 1e-5

_CACHE = {}


def _pbcast_src(tile_ap: AP, part_row: int, part_pitch: int, dims, offset_elems: int):
    """Manual AP: read from partition `part_row` of an SBUF tile, broadcast
    across 64 partitions (leading 0-stride dim), with free dims `dims`
    (list of [step, count]) starting at byte/elem offset `offset_elems`."""
    base = tile_ap  # AP covering whole tile, standard layout
    # Flat-element convention: partition p lives at p * part_pitch.
    off = part_row * part_pitch + offset_elems
    return AP(base.tensor, base.offset + off, [[0, 64]] + list(dims))


def build_graph():
    nc = bacc.Bacc(None, target_bir_lowering=False)

    xs = nc.declare_dram_parameter("xs", [IMGS, C, H, W], BF16, isOutput=False)
    w1p = nc.declare_dram_parameter("w1p", [128, 3, INTER], BF16, isOutput=False)
    w1s = nc.declare_dram_parameter("w1s", [C, 3, INTER], BF16, isOutput=False)
    s1 = nc.declare_dram_parameter("s1", [INTER, 1], F32, isOutput=False)
    b1 = nc.declare_dram_parameter("b1", [INTER, 1], F32, isOutput=False)
    w2p = nc.declare_dram_parameter("w2p", [128, 3, BCH], BF16, isOutput=False)
    w2s = nc.declare_dram_parameter("w2s", [INTER, 3, BCH], BF16, isOutput=False)
    s2 = nc.declare_dram_parameter("s2", [BCH, 1], F32, isOutput=False)
    b2 = nc.declare_dram_parameter("b2", [BCH, 1], F32, isOutput=False)
    wpbp = nc.declare_dram_parameter("wpbp", [128, 3, NT * C], BF16, isOutput=False)
    wpbs = nc.declare_dram_parameter("wpbs", [C, 3, NT * C], BF16, isOutput=False)
    coefp = nc.declare_dram_parameter("coefp", [128, NF, O], BF16, isOutput=False)
    out = nc.declare_dram_parameter("out", [IMGS, O, W, H], F32, isOutput=True)

    with tile.TileContext(nc) as tc:
        with (
            tc.tile_pool(name="persist", bufs=1) as pp,
            tc.tile_pool(name="bcrep", bufs=3) as bp,
            tc.tile_pool(name="uu", bufs=2) as up,
            tc.tile_pool(name="tmp", bufs=2) as tp,
            tc.tile_pool(name="osb", bufs=2) as op_,
            tc.tile_pool(name="dramb", bufs=1, space=bass.MemorySpace.DRAM) as dp,
            tc.tile_pool(name="ps_conv", bufs=1, space=bass.MemorySpace.PSUM) as pcv,
            tc.tile_pool(name="ps_pb", bufs=1, space=bass.MemorySpace.PSUM) as ppb,
            tc.tile_pool(name="ps_fin", bufs=1, space=bass.MemorySpace.PSUM) as pfn,
        ):
            # ---- persistent SBUF ----
            w1psb = pp.tile([128, 3, INTER], BF16, tag="w1psb")
            w1ssb = pp.tile([C, 3, INTER], BF16, tag="w1ssb")
            w2psb = pp.tile([128, 3, BCH], BF16, tag="w2psb")
            w2ssb = pp.tile([INTER, 3, BCH], BF16, tag="w2ssb")
            wpbpsb = pp.tile([128, 3, NT * C], BF16, tag="wpbpsb")
            wpbssb = pp.tile([C, 3, NT * C], BF16, tag="wpbssb")
            coefsb = pp.tile([128, NF, O], BF16, tag="coefsb")
            s1sb = pp.tile([INTER, 1], F32, tag="s1sb")
            b1sb = pp.tile([INTER, 1], F32, tag="b1sb")
            s2sb = pp.tile([BCH, 1], F32, tag="s2sb")
            b2sb = pp.tile([BCH, 1], F32, tag="b2sb")
            nc.sync.dma_start(w1psb[:], w1p[:])
            nc.sync.dma_start(w1ssb[:], w1s[:])
            nc.sync.dma_start(w2psb[:], w2p[:])
            nc.sync.dma_start(w2ssb[:], w2s[:])
            nc.sync.dma_start(wpbpsb[:], wpbp[:])
            nc.sync.dma_start(wpbssb[:], wpbs[:])
            nc.sync.dma_start(coefsb[:], coefp[:])
            nc.sync.dma_start(s1sb[:], s1[:])
            nc.sync.dma_start(b1sb[:], b1[:])
            nc.sync.dma_start(s2sb[:], s2[:])
            nc.sync.dma_start(b2sb[:], b2[:])

            xpad = []
            hpad = []
            bcs = []
            for i in range(IMGS):
                xp = pp.tile([128, HP, HP], BF16, tag=f"xpad{i}", name=f"xpad{i}")
                hp = pp.tile([128, HP, HP], BF16, tag=f"hpad{i}", name=f"hpad{i}")
                bi = pp.tile([BCH, H, W], BF16, tag=f"bc{i}", name=f"bc{i}")
                nc.vector.memset(xp[:], 0.0)
                nc.vector.memset(hp[:], 0.0)
                nc.sync.dma_start(xp[0:64, 1 : H + 1, 1 : W + 1], xs[i])
                nc.sync.dma_start(xp[64:128, 1 : H + 1, 0:W], xs[i])
                xpad.append(xp)
                hpad.append(hp)
                bcs.append(bi)
            pbt = [pp.tile([128, Q], BF16, tag=f"pbt{t}", name=f"pbt{t}") for t in range(NT)]
            bcd = [
                dp.tile([NCHUNK, BCH, CH], BF16, tag=f"bcd{i}", name=f"bcd{i}")
                for i in range(IMGS)
            ]

            # ---- stage 1+2+PB per image ----
            for i in range(IMGS):
                # conv1 -> bn -> tanh -> hpad
                for g in range(2):  # chunk groups of 4 (PSUM budget)
                    cps = pcv.tile([INTER, 4, RC, W], F32, tag="convps")
                    for m in range(6):
                        ki = m % 3
                        pair = m < 3
                        lhsT = w1psb[:, ki, :] if pair else w1ssb[:, ki, :]
                        for j in range(4):
                            h0 = (g * 4 + j) * RC
                            if pair:  # taps (ki,0)+(ki,1): hi half pre-shifted
                                rhs = xpad[i][:, h0 + ki : h0 + ki + RC, 0:W]
                            else:     # tap (ki,2)
                                rhs = xpad[i][0:64, h0 + ki : h0 + ki + RC, 2 : 2 + W]
                            nc.tensor.matmul(
                                cps[:, j], lhsT, rhs, start=(m == 0), stop=(m == 5)
                            )
                    for j in range(4):
                        h0 = (g * 4 + j) * RC
                        nc.scalar.activation(
                            hpad[i][0:64, h0 + 1 : h0 + 1 + RC, 1 : W + 1],
                            cps[:, j],
                            AF.Tanh,
                            bias=b1sb[:],
                            scale=s1sb[:],
                        )
                        nc.scalar.activation(
                            hpad[i][64:128, h0 + 1 : h0 + 1 + RC, 0:W],
                            cps[:, j],
                            AF.Tanh,
                            bias=b1sb[:],
                            scale=s1sb[:],
                        )
                # conv2 -> bn -> tanh -> bc
                for g in range(2):
                    cps = pcv.tile([BCH, 4, RC, W], F32, tag="convps")
                    for m in range(6):
                        ki = m % 3
                        pair = m < 3
                        lhsT = w2psb[:, ki, :] if pair else w2ssb[:, ki, :]
                        for j in range(4):
                            h0 = (g * 4 + j) * RC
                            if pair:
                                rhs = hpad[i][:, h0 + ki : h0 + ki + RC, 0:W]
                            else:
                                rhs = hpad[i][0:64, h0 + ki : h0 + ki + RC, 2 : 2 + W]
                            nc.tensor.matmul(
                                cps[:, j], lhsT, rhs, start=(m == 0), stop=(m == 5)
                            )
                    for j in range(4):
                        h0 = (g * 4 + j) * RC
                        # transposed store: bcT[ch, w_out, h_bc] so later
                        # per-pixel reads are contiguous
                        nc.scalar.activation(
                            bcs[i][:, :, h0 : h0 + RC].transpose([0, 2, 1]),
                            cps[:, j],
                            AF.Tanh,
                            bias=b2sb[:],
                            scale=s2sb[:],
                        )
            for i in range(IMGS):
                for ch in range(NCHUNK):
                    nc.sync.dma_start(
                        bcd[i][ch], bcs[i][:, ch * RC : (ch + 1) * RC, :].opt()
                    )

            # ---- per chunk: PB (PE) -> per-pixel bilinear (DVE+Pool) -> final ----
            bc_pitch = H * W
            POOL_F = (11, 12, 13, 14, 15)  # f values computed on GpSimd
            for ch in range(NCHUNK):
                h0 = ch * RC
                c0 = h0 * W
                for i in range(IMGS):
                    pps = ppb.tile([128, 3, RC, W], F32, tag="pbps")
                    for j3 in range(3):
                        for m in range(6):
                            ki = m % 3
                            pair = m < 3
                            if pair:
                                lhsT = wpbpsb[:, ki, j3 * 128 : (j3 + 1) * 128]
                                rhs = xpad[i][:, h0 + ki : h0 + ki + RC, 0:W]
                            else:
                                lhsT = wpbssb[:, ki, j3 * 128 : (j3 + 1) * 128]
                                rhs = xpad[i][0:64, h0 + ki : h0 + ki + RC, 2 : 2 + W]
                            nc.tensor.matmul(
                                pps[:, j3], lhsT, rhs, start=(m == 0), stop=(m == 5)
                            )
                    for j3 in range(3):
                        nc.scalar.copy(
                            pbt[2 * j3][i * 64 : (i + 1) * 64, c0 : c0 + CH],
                            pps[0:64, j3].opt(),
                        )
                        nc.scalar.copy(
                            pbt[2 * j3 + 1][i * 64 : (i + 1) * 64, c0 : c0 + CH],
                            pps[64:128, j3].opt(),
                        )
                uu_v = up.tile([128, 11 * CH], BF16, tag="uuv")
                uu_p = up.tile([128, 5 * CH], BF16, tag="uup")
                for f in range(NF):
                    use_pool_f = f in POOL_F
                    eng = nc.gpsimd if use_pool_f else nc.vector
                    btag = "bcrp" if use_pool_f else "bcrv"
                    bcr = bp.tile(
                        [128, NT * CH], BF16, tag=btag, name=btag,
                        bufs=3 if use_pool_f else 4,
                    )
                    bcr_full = bcr[:]
                    for i in range(IMGS):
                        dfull = bcd[i][:]
                        srcap = AP(
                            dfull.tensor,
                            dfull.offset + ch * BCH * CH + f * NT * CH,
                            [[0, 64], [1, NT * CH]],
                        )
                        dstap = AP(
                            bcr_full.tensor,
                            bcr_full.offset + i * 64 * (NT * CH),
                            [[NT * CH, 64], [1, NT * CH]],
                        )
                        iss = nc.sync if (f + i) % 2 == 0 else nc.scalar
                        iss.dma_start(dstap, srcap)
                    use_pool = f in POOL_F
                    if use_pool:
                        fi = POOL_F.index(f)
                        uslice = uu_p[:, fi * CH : (fi + 1) * CH]
                    else:
                        uslice = uu_v[:, (f - sum(1 for p in POOL_F if p < f)) * CH :][:, :CH]
                    for t in range(NT):
                        pbin = pbt[t][:, c0 : c0 + CH]
                        bslice = bcr[:, t * CH : (t + 1) * CH]
                        if t == 0:
                            eng.tensor_mul(uslice, pbin, bslice)
                        else:
                            tag = "tmpp" if use_pool else "tmpv"
                            tmp = tp.tile([128, CH], BF16, tag=tag, name=tag)
                            eng.tensor_mul(tmp[:], pbin, bslice)
                            eng.tensor_add(uslice, tmp[:], uslice)
                # final 1x1: out[o, q] = sum_{c,f} coefp * u
                for i in range(IMGS):
                    fps = pfn.tile([O, RC, W], F32, tag="finps")
                    for f in range(NF):
                        if f in POOL_F:
                            fi = POOL_F.index(f)
                            usl = uu_p[i * 64 : (i + 1) * 64, fi * CH : (fi + 1) * CH]
                        else:
                            fv = f - sum(1 for p in POOL_F if p < f)
                            usl = uu_v[i * 64 : (i + 1) * 64, fv * CH : (fv + 1) * CH]
                        nc.tensor.matmul(
                            fps[:],
                            coefsb[i * 64 : (i + 1) * 64, f, :],
                            usl,
                            start=(f == 0),
                            stop=(f == NF - 1),
                        )
                    osb = op_.tile([O, W, RC], F32, tag="osb")
                    # store transposed: osb[o, w, h]
                    nc.scalar.copy(osb[:].transpose([0, 2, 1]), fps[:])
                    nc.sync.dma_start(out[i, :, :, h0 : h0 + RC], osb[:])

    nc.compile()
    return nc


def _prep_params(inputs):
    bf16 = ml_dtypes.bfloat16
    f32 = np.float32
    c1w = np.asarray(inputs["conv1_w"], f32)
    c2w = np.asarray(inputs["conv2_w"], f32)
    bases = np.asarray(inputs["bases"], f32)
    coef = np.asarray(inputs["coef"], f32)

    s1 = np.asarray(inputs["bn1_gamma"], f32) / np.sqrt(
        np.asarray(inputs["bn1_var"], f32) + BN_EPS
    )
    b1 = (np.asarray(inputs["conv1_b"], f32) - np.asarray(inputs["bn1_mean"], f32)) * s1 + np.asarray(
        inputs["bn1_beta"], f32
    )
    s2 = np.asarray(inputs["bn2_gamma"], f32) / np.sqrt(
        np.asarray(inputs["bn2_var"], f32) + BN_EPS
    )
    b2 = (np.asarray(inputs["conv2_b"], f32) - np.asarray(inputs["bn2_mean"], f32)) * s2 + np.asarray(
        inputs["bn2_beta"], f32
    )

    w1pk = np.zeros((128, 3, INTER), f32)
    w1sk = np.zeros((C, 3, INTER), f32)
    w2pk = np.zeros((128, 3, BCH), f32)
    w2sk = np.zeros((INTER, 3, BCH), f32)
    for ki in range(3):
        w1pk[0:64, ki] = c1w[:, :, ki, 0].T
        w1pk[64:128, ki] = c1w[:, :, ki, 1].T
        w1sk[:, ki] = c1w[:, :, ki, 2].T
        w2pk[0:64, ki] = c2w[:, :, ki, 0].T
        w2pk[64:128, ki] = c2w[:, :, ki, 1].T
        w2sk[:, ki] = c2w[:, :, ki, 2].T

    wpbpk = np.zeros((128, 3, NT * C), f32)
    wpbsk = np.zeros((C, 3, NT * C), f32)
    for t in range(NT):
        for c in range(C):
            for ki in range(3):
                wpbpk[c, ki, t * C + c] = bases[t, 3 * ki + 0]
                wpbpk[64 + c, ki, t * C + c] = bases[t, 3 * ki + 1]
                wpbsk[c, ki, t * C + c] = bases[t, 3 * ki + 2]

    cview = coef.reshape(O, C, NF)  # coef[o, 16c+f]
    coefp64 = np.ascontiguousarray(cview.transpose(1, 2, 0))  # [c, f, o]
    coefp = np.concatenate([coefp64, coefp64], axis=0)  # [128, f, o]

    return {
        "w1p": w1pk.astype(bf16),
        "w1s": w1sk.astype(bf16),
        "s1": s1.reshape(-1, 1).astype(f32),
        "b1": b1.reshape(-1, 1).astype(f32),
        "w2p": w2pk.astype(bf16),
        "w2s": w2sk.astype(bf16),
        "s2": s2.reshape(-1, 1).astype(f32),
        "b2": b2.reshape(-1, 1).astype(f32),
        "wpbp": wpbpk.astype(bf16),
        "wpbs": wpbsk.astype(bf16),
        "coefp": coefp.astype(bf16),
    }


def kernel(**inputs):
    if "nc" not in _CACHE:
        _CACHE["nc"] = build_graph()
    nc = _CACHE["nc"]

    params = _prep_params(inputs)
    x = np.asarray(inputs["x"], np.float32).astype(ml_dtypes.bfloat16)

    in_maps = []
    for core in range(N_CORES):
        m = dict(params)
        m["xs"] = np.ascontiguousarray(x[core * IMGS : (core + 1) * IMGS])
        in_maps.append(m)

    res = run_bass_kernel_spmd(nc, in_maps, core_ids=list(range(N_CORES)))
    outs = [r["out"] for r in res.results]
    return np.concatenate(outs, axis=0).astype(np.float32)


def _install_ntff_hook():
    """Shim antenv.axon_hooks with the trn_boot ctypes NTFF hook."""
    import types

    try:
        from antenv.axon_hooks import get_axon_ntff_profile_hook  # noqa
        return
    except ImportError:
        pass
    sys.path.insert(0, "/root/.axon_site/trn_agent_boot")
    import trn_boot

    hook = trn_boot._ntff_profile_via_ctypes("/opt/axon/libaxon_pjrt.so")
    mod_pkg = sys.modules.get("antenv")
    if mod_pkg is None:
        mod_pkg = types.ModuleType("antenv")
        sys.modules["antenv"] = mod_pkg
    mod = types.ModuleType("antenv.axon_hooks")
    mod.get_axon_ntff_profile_hook = lambda: hook
    mod.set_axon_ntff_profile_hook = lambda h: None
    sys.modules["antenv.axon_hooks"] = mod
    mod_pkg.axon_hooks = mod


def run_timed(inputs):
    """Run once with NTFF tracing; return exec_time_ns (or None)."""
    _install_ntff_hook()
    if "nc" not in _CACHE:
        _CACHE["nc"] = build_graph()
    nc = _CACHE["nc"]
    params = _prep_params(inputs)
    x = np.asarray(inputs["x"], np.float32).astype(ml_dtypes.bfloat16)
    in_maps = []
    for core in range(N_CORES):
        m = dict(params)
        m["xs"] = np.ascontiguousarray(x[core * IMGS : (core + 1) * IMGS])
        in_maps.append(m)
    res = run_bass_kernel_spmd(
        nc, in_maps, core_ids=list(range(N_CORES)), trace=True
    )
    print("trace profile_json:", res.profile_json)
    _CACHE["last_res"] = res
    return res.exec_time_ns


if __name__ == "__main__":
    rng = np.random.default_rng(0)
    fake = {
        "x": rng.standard_normal((16, 64, 64, 64), np.float32),
        "conv1_w": rng.standard_normal((64, 64, 3, 3), np.float32) * 0.05,
        "conv1_b": rng.standard_normal((64,), np.float32) * 0.05,
        "bn1_gamma": rng.uniform(0.5, 1.5, (64,)).astype(np.float32),
        "bn1_beta": rng.standard_normal((64,), np.float32) * 0.05,
        "bn1_mean": rng.standard_normal((64,), np.float32) * 0.05,
        "bn1_var": rng.uniform(0.5, 1.5, (64,)).astype(np.float32),
        "conv2_w": rng.standard_normal((96, 64, 3, 3), np.float32) * 0.05,
        "conv2_b": rng.standard_normal((96,), np.float32) * 0.05,
        "bn2_gamma": rng.uniform(0.5, 1.5, (96,)).astype(np.float32),
        "bn2_beta": rng.standard_normal((96,), np.float32) * 0.05,
        "bn2_mean": rng.standard_normal((96,), np.float32) * 0.05,
        "bn2_var": rng.uniform(0.5, 1.5, (96,)).astype(np.float32),
        "bases": rng.standard_normal((6, 9), np.float32),
        "coef": rng.standard_normal((128, 1024), np.float32) * 0.02,
    }
    o = kernel(**fake)
    print("out", o.shape, o.dtype)


# revision 18
# speedup vs baseline: 1.7165x; 1.1966x over previous
"""Trainium2 Bass kernel for nn_ADConv (adaptive-basis conv).

Math (per image, per pixel q=(h,w)):
  h1  = tanh(bn1(conv3x3(x)))                      # [64, H, W]
  bc  = tanh(bn2(conv3x3(h1)))                     # [96, H, W], channel = 6f+t
  PB[c,t,q]   = sum_k x[c, q+dk] * B[t,k]          # depthwise basis conv
  u[c,f,q]    = sum_t PB[c,t,q] * bc[6f+t, wq, hq] # per-pixel bilinear (DVE)
  out[o,w,h]  = sum_{c,f} coef[o, 16c+f] * u[c,f,q]

Sharding: data-parallel, batch 16 -> 2 images per NeuronCore, params
replicated. Everything computed in bf16 (fp32 PSUM accumulation).
"""

import os
import sys

import numpy as np

sys.path.insert(0, "/opt/trn_rl_repo")

import ml_dtypes

import concourse.bacc as bacc
import concourse.bass as bass
import concourse.mybir as mybir
import concourse.tile as tile
from concourse.ap import AP
from concourse.bass_utils import run_bass_kernel_spmd

BF16 = mybir.dt.bfloat16
F32 = mybir.dt.float32
AF = mybir.ActivationFunctionType
ALU = mybir.AluOpType

N_CORES = 8
IMGS = 2           # images per core
C = 64             # input channels
INTER = 64         # conv1 out channels
BCH = 96           # conv2 out channels = 16f * 6t
NT = 6             # TOTAL_BASES
NF = 16            # NUM_FA
O = 128            # output channels
H = W = 64
HP = 66            # padded spatial
Q = H * W          # 4096 pixels
RC = 8             # rows per chunk
NCHUNK = H // RC   # 8 chunks of 512 px
CH = RC * W        # 512 px per chunk
BN_EPS = 1e-5

_CACHE = {}


def _pbcast_src(tile_ap: AP, part_row: int, part_pitch: int, dims, offset_elems: int):
    """Manual AP: read from partition `part_row` of an SBUF tile, broadcast
    across 64 partitions (leading 0-stride dim), with free dims `dims`
    (list of [step, count]) starting at byte/elem offset `offset_elems`."""
    base = tile_ap  # AP covering whole tile, standard layout
    # Flat-element convention: partition p lives at p * part_pitch.
    off = part_row * part_pitch + offset_elems
    return AP(base.tensor, base.offset + off, [[0, 64]] + list(dims))


def build_graph():
    nc = bacc.Bacc(None, target_bir_lowering=False)

    xs = nc.declare_dram_parameter("xs", [IMGS, C, H, W], BF16, isOutput=False)
    w1p = nc.declare_dram_parameter("w1p", [128, 3, INTER], BF16, isOutput=False)
    w1s = nc.declare_dram_parameter("w1s", [C, 3, INTER], BF16, isOutput=False)
    s1 = nc.declare_dram_parameter("s1", [INTER, 1], F32, isOutput=False)
    b1 = nc.declare_dram_parameter("b1", [INTER, 1], F32, isOutput=False)
    w2p = nc.declare_dram_parameter("w2p", [128, 3, BCH], BF16, isOutput=False)
    w2s = nc.declare_dram_parameter("w2s", [INTER, 3, BCH], BF16, isOutput=False)
    s2 = nc.declare_dram_parameter("s2", [BCH, 1], F32, isOutput=False)
    b2 = nc.declare_dram_parameter("b2", [BCH, 1], F32, isOutput=False)
    wpbp = nc.declare_dram_parameter("wpbp", [128, 3, NT * C], BF16, isOutput=False)
    wpbs = nc.declare_dram_parameter("wpbs", [C, 3, NT * C], BF16, isOutput=False)
    coefp = nc.declare_dram_parameter("coefp", [128, NF, O], BF16, isOutput=False)
    out = nc.declare_dram_parameter("out", [IMGS, O, W, H], F32, isOutput=True)

    with tile.TileContext(nc) as tc:
        with (
            tc.tile_pool(name="persist", bufs=1) as pp,
            tc.tile_pool(name="bcrep", bufs=3) as bp,
            tc.tile_pool(name="uu", bufs=2) as up,
            tc.tile_pool(name="tmp", bufs=2) as tp,
            tc.tile_pool(name="osb", bufs=2) as op_,
            tc.tile_pool(name="dramb", bufs=1, space=bass.MemorySpace.DRAM) as dp,
            tc.tile_pool(name="ps_conv", bufs=1, space=bass.MemorySpace.PSUM) as pcv,
            tc.tile_pool(name="ps_pb", bufs=1, space=bass.MemorySpace.PSUM) as ppb,
            tc.tile_pool(name="ps_fin", bufs=1, space=bass.MemorySpace.PSUM) as pfn,
        ):
            # ---- persistent SBUF ----
            w1psb = pp.tile([128, 3, INTER], BF16, tag="w1psb")
            w1ssb = pp.tile([C, 3, INTER], BF16, tag="w1ssb")
            w2psb = pp.tile([128, 3, BCH], BF16, tag="w2psb")
            w2ssb = pp.tile([INTER, 3, BCH], BF16, tag="w2ssb")
            wpbpsb = pp.tile([128, 3, NT * C], BF16, tag="wpbpsb")
            wpbssb = pp.tile([C, 3, NT * C], BF16, tag="wpbssb")
            coefsb = pp.tile([128, NF, O], BF16, tag="coefsb")
            s1sb = pp.tile([INTER, 1], F32, tag="s1sb")
            b1sb = pp.tile([INTER, 1], F32, tag="b1sb")
            s2sb = pp.tile([BCH, 1], F32, tag="s2sb")
            b2sb = pp.tile([BCH, 1], F32, tag="b2sb")
            nc.sync.dma_start(w1psb[:], w1p[:])
            nc.sync.dma_start(w1ssb[:], w1s[:])
            nc.sync.dma_start(w2psb[:], w2p[:])
            nc.sync.dma_start(w2ssb[:], w2s[:])
            nc.sync.dma_start(wpbpsb[:], wpbp[:])
            nc.sync.dma_start(wpbssb[:], wpbs[:])
            nc.sync.dma_start(coefsb[:], coefp[:])
            nc.sync.dma_start(s1sb[:], s1[:])
            nc.sync.dma_start(b1sb[:], b1[:])
            nc.sync.dma_start(s2sb[:], s2[:])
            nc.sync.dma_start(b2sb[:], b2[:])

            xpad = []
            hpad = []
            bcs = []
            for i in range(IMGS):
                xp = pp.tile([128, HP, HP], BF16, tag=f"xpad{i}", name=f"xpad{i}")
                hp = pp.tile([128, HP, HP], BF16, tag=f"hpad{i}", name=f"hpad{i}")
                bi = pp.tile([BCH, H, W], BF16, tag=f"bc{i}", name=f"bc{i}")
                nc.vector.memset(xp[:], 0.0)
                nc.vector.memset(hp[:], 0.0)
                nc.sync.dma_start(xp[0:64, 1 : H + 1, 1 : W + 1], xs[i])
                nc.sync.dma_start(xp[64:128, 1 : H + 1, 0:W], xs[i])
                xpad.append(xp)
                hpad.append(hp)
                bcs.append(bi)
            pbt = [pp.tile([128, Q], BF16, tag=f"pbt{t}", name=f"pbt{t}") for t in range(NT)]
            bcd = [
                dp.tile([NCHUNK, BCH, CH], BF16, tag=f"bcd{i}", name=f"bcd{i}")
                for i in range(IMGS)
            ]

            # ---- stage 1+2+PB per image ----
            for i in range(IMGS):
                # conv1 -> bn -> tanh -> hpad
                for g in range(4):  # chunk groups of 2 (PSUM budget)
                    cps = pcv.tile([INTER, 2, RC, W], F32, tag="convps")
                    for m in range(6):
                        ki = m % 3
                        pair = m < 3
                        lhsT = w1psb[:, ki, :] if pair else w1ssb[:, ki, :]
                        for j in range(2):
                            h0 = (g * 2 + j) * RC
                            if pair:  # taps (ki,0)+(ki,1): hi half pre-shifted
                                rhs = xpad[i][:, h0 + ki : h0 + ki + RC, 0:W]
                            else:     # tap (ki,2)
                                rhs = xpad[i][0:64, h0 + ki : h0 + ki + RC, 2 : 2 + W]
                            nc.tensor.matmul(
                                cps[:, j], lhsT, rhs, start=(m == 0), stop=(m == 5)
                            )
                    for j in range(2):
                        h0 = (g * 2 + j) * RC
                        nc.scalar.activation(
                            hpad[i][0:64, h0 + 1 : h0 + 1 + RC, 1 : W + 1],
                            cps[:, j],
                            AF.Tanh,
                            bias=b1sb[:],
                            scale=s1sb[:],
                        )
                        nc.scalar.activation(
                            hpad[i][64:128, h0 + 1 : h0 + 1 + RC, 0:W],
                            cps[:, j],
                            AF.Tanh,
                            bias=b1sb[:],
                            scale=s1sb[:],
                        )
                # conv2 -> bn -> tanh -> bc
                for g in range(4):
                    cps = pcv.tile([BCH, 2, RC, W], F32, tag="convps")
                    for m in range(6):
                        ki = m % 3
                        pair = m < 3
                        lhsT = w2psb[:, ki, :] if pair else w2ssb[:, ki, :]
                        for j in range(2):
                            h0 = (g * 2 + j) * RC
                            if pair:
                                rhs = hpad[i][:, h0 + ki : h0 + ki + RC, 0:W]
                            else:
                                rhs = hpad[i][0:64, h0 + ki : h0 + ki + RC, 2 : 2 + W]
                            nc.tensor.matmul(
                                cps[:, j], lhsT, rhs, start=(m == 0), stop=(m == 5)
                            )
                    for j in range(2):
                        h0 = (g * 2 + j) * RC
                        # transposed store: bcT[ch, w_out, h_bc] so later
                        # per-pixel reads are contiguous
                        nc.scalar.activation(
                            bcs[i][:, :, h0 : h0 + RC].transpose([0, 2, 1]),
                            cps[:, j],
                            AF.Tanh,
                            bias=b2sb[:],
                            scale=s2sb[:],
                        )
            for i in range(IMGS):
                for ch in range(NCHUNK):
                    nc.sync.dma_start(
                        bcd[i][ch], bcs[i][:, ch * RC : (ch + 1) * RC, :].opt()
                    )

            # ---- per chunk: PB (PE) -> products (DVE) -> final w/ t-fold (PE) ----
            # u-chunks of 1024 px (2 PB chunks each)
            UCH = 2 * CH  # 1024
            for uc in range(NCHUNK // 2):
                for half in range(2):
                    ch = uc * 2 + half
                    h0 = ch * RC
                    c0 = h0 * W
                    for i in range(IMGS):
                        for j3 in range(3):
                            pps = ppb.tile(
                                [128, RC, W], F32, tag="pbps", name="pps", bufs=2
                            )
                            for m in range(6):
                                ki = m % 3
                                pair = m < 3
                                if pair:
                                    lhsT = wpbpsb[:, ki, j3 * 128 : (j3 + 1) * 128]
                                    rhs = xpad[i][:, h0 + ki : h0 + ki + RC, 0:W]
                                else:
                                    lhsT = wpbssb[:, ki, j3 * 128 : (j3 + 1) * 128]
                                    rhs = xpad[i][0:64, h0 + ki : h0 + ki + RC, 2 : 2 + W]
                                nc.tensor.matmul(
                                    pps[:], lhsT, rhs, start=(m == 0), stop=(m == 5)
                                )
                            nc.scalar.copy(
                                pbt[2 * j3][i * 64 : (i + 1) * 64, c0 : c0 + CH],
                                pps[0:64].opt(),
                            )
                            nc.scalar.copy(
                                pbt[2 * j3 + 1][i * 64 : (i + 1) * 64, c0 : c0 + CH],
                                pps[64:128].opt(),
                            )
                u0 = uc * UCH
                fps = [
                    pfn.tile([O, 2, CH], F32, tag=f"finps{i}", name=f"fps{i}", bufs=1)
                    for i in range(IMGS)
                ]
                for f in range(NF):
                    bcr = bp.tile([128, NT * UCH], BF16, tag="bcrv", name="bcrv", bufs=3)
                    bcr_full = bcr[:]
                    for i in range(IMGS):
                        dfull = bcd[i][:]
                        # two PB-chunks back to back in chunk-major bcd
                        for half in range(2):
                            srcap = AP(
                                dfull.tensor,
                                dfull.offset
                                + (uc * 2 + half) * BCH * CH
                                + f * NT * CH,
                                [[0, 64], [1, NT * CH]],
                            )
                            dstap = AP(
                                bcr_full.tensor,
                                bcr_full.offset
                                + i * 64 * (NT * UCH)
                                + half * CH,
                                [[NT * UCH, 64], [UCH, NT], [1, CH]],
                            )
                            iss = nc.sync if (f + i + half) % 2 == 0 else nc.scalar
                            iss.dma_start(dstap, srcap)
                    theta = tp.tile([128, NT * UCH], BF16, tag="theta", name="theta", bufs=3)
                    for t in range(NT):
                        nc.vector.tensor_mul(
                            theta[:, t * UCH : (t + 1) * UCH],
                            pbt[t][:, u0 : u0 + UCH],
                            bcr[:, t * UCH : (t + 1) * UCH],
                        )
                    # fold t-sum into final accumulation: 6 mms per (f, i, half)
                    for i in range(IMGS):
                        for t in range(NT):
                            for half in range(2):
                                nc.tensor.matmul(
                                    fps[i][:, half],
                                    coefsb[i * 64 : (i + 1) * 64, f, :],
                                    theta[
                                        i * 64 : (i + 1) * 64,
                                        t * UCH + half * CH : t * UCH + (half + 1) * CH,
                                    ],
                                    start=(f == 0 and t == 0),
                                    stop=(f == NF - 1 and t == NT - 1),
                                )
                for i in range(IMGS):
                    for half in range(2):
                        ch = uc * 2 + half
                        h0 = ch * RC
                        osb = op_.tile([O, W, RC], F32, tag="osb", name="osb")
                        # store transposed: osb[o, w, h]
                        nc.scalar.copy(
                            osb[:].transpose([0, 2, 1]),
                            fps[i][:, half].opt(),
                        )
                        nc.sync.dma_start(out[i, :, :, h0 : h0 + RC], osb[:])

    nc.compile()
    return nc


def _prep_params(inputs):
    bf16 = ml_dtypes.bfloat16
    f32 = np.float32
    c1w = np.asarray(inputs["conv1_w"], f32)
    c2w = np.asarray(inputs["conv2_w"], f32)
    bases = np.asarray(inputs["bases"], f32)
    coef = np.asarray(inputs["coef"], f32)

    s1 = np.asarray(inputs["bn1_gamma"], f32) / np.sqrt(
        np.asarray(inputs["bn1_var"], f32) + BN_EPS
    )
    b1 = (np.asarray(inputs["conv1_b"], f32) - np.asarray(inputs["bn1_mean"], f32)) * s1 + np.asarray(
        inputs["bn1_beta"], f32
    )
    s2 = np.asarray(inputs["bn2_gamma"], f32) / np.sqrt(
        np.asarray(inputs["bn2_var"], f32) + BN_EPS
    )
    b2 = (np.asarray(inputs["conv2_b"], f32) - np.asarray(inputs["bn2_mean"], f32)) * s2 + np.asarray(
        inputs["bn2_beta"], f32
    )

    w1pk = np.zeros((128, 3, INTER), f32)
    w1sk = np.zeros((C, 3, INTER), f32)
    w2pk = np.zeros((128, 3, BCH), f32)
    w2sk = np.zeros((INTER, 3, BCH), f32)
    for ki in range(3):
        w1pk[0:64, ki] = c1w[:, :, ki, 0].T
        w1pk[64:128, ki] = c1w[:, :, ki, 1].T
        w1sk[:, ki] = c1w[:, :, ki, 2].T
        w2pk[0:64, ki] = c2w[:, :, ki, 0].T
        w2pk[64:128, ki] = c2w[:, :, ki, 1].T
        w2sk[:, ki] = c2w[:, :, ki, 2].T

    wpbpk = np.zeros((128, 3, NT * C), f32)
    wpbsk = np.zeros((C, 3, NT * C), f32)
    for t in range(NT):
        for c in range(C):
            for ki in range(3):
                wpbpk[c, ki, t * C + c] = bases[t, 3 * ki + 0]
                wpbpk[64 + c, ki, t * C + c] = bases[t, 3 * ki + 1]
                wpbsk[c, ki, t * C + c] = bases[t, 3 * ki + 2]

    cview = coef.reshape(O, C, NF)  # coef[o, 16c+f]
    coefp64 = np.ascontiguousarray(cview.transpose(1, 2, 0))  # [c, f, o]
    coefp = np.concatenate([coefp64, coefp64], axis=0)  # [128, f, o]

    return {
        "w1p": w1pk.astype(bf16),
        "w1s": w1sk.astype(bf16),
        "s1": s1.reshape(-1, 1).astype(f32),
        "b1": b1.reshape(-1, 1).astype(f32),
        "w2p": w2pk.astype(bf16),
        "w2s": w2sk.astype(bf16),
        "s2": s2.reshape(-1, 1).astype(f32),
        "b2": b2.reshape(-1, 1).astype(f32),
        "wpbp": wpbpk.astype(bf16),
        "wpbs": wpbsk.astype(bf16),
        "coefp": coefp.astype(bf16),
    }


def kernel(**inputs):
    if "nc" not in _CACHE:
        _CACHE["nc"] = build_graph()
    nc = _CACHE["nc"]

    params = _prep_params(inputs)
    x = np.asarray(inputs["x"], np.float32).astype(ml_dtypes.bfloat16)

    in_maps = []
    for core in range(N_CORES):
        m = dict(params)
        m["xs"] = np.ascontiguousarray(x[core * IMGS : (core + 1) * IMGS])
        in_maps.append(m)

    res = run_bass_kernel_spmd(nc, in_maps, core_ids=list(range(N_CORES)))
    outs = [r["out"] for r in res.results]
    return np.concatenate(outs, axis=0).astype(np.float32)


def _install_ntff_hook():
    """Shim antenv.axon_hooks with the trn_boot ctypes NTFF hook."""
    import types

    try:
        from antenv.axon_hooks import get_axon_ntff_profile_hook  # noqa
        return
    except ImportError:
        pass
    sys.path.insert(0, "/root/.axon_site/trn_agent_boot")
    import trn_boot

    hook = trn_boot._ntff_profile_via_ctypes("/opt/axon/libaxon_pjrt.so")
    mod_pkg = sys.modules.get("antenv")
    if mod_pkg is None:
        mod_pkg = types.ModuleType("antenv")
        sys.modules["antenv"] = mod_pkg
    mod = types.ModuleType("antenv.axon_hooks")
    mod.get_axon_ntff_profile_hook = lambda: hook
    mod.set_axon_ntff_profile_hook = lambda h: None
    sys.modules["antenv.axon_hooks"] = mod
    mod_pkg.axon_hooks = mod


def run_timed(inputs):
    """Run once with NTFF tracing; return exec_time_ns (or None)."""
    _install_ntff_hook()
    if "nc" not in _CACHE:
        _CACHE["nc"] = build_graph()
    nc = _CACHE["nc"]
    params = _prep_params(inputs)
    x = np.asarray(inputs["x"], np.float32).astype(ml_dtypes.bfloat16)
    in_maps = []
    for core in range(N_CORES):
        m = dict(params)
        m["xs"] = np.ascontiguousarray(x[core * IMGS : (core + 1) * IMGS])
        in_maps.append(m)
    res = run_bass_kernel_spmd(
        nc, in_maps, core_ids=list(range(N_CORES)), trace=True
    )
    print("trace profile_json:", res.profile_json)
    _CACHE["last_res"] = res
    return res.exec_time_ns


if __name__ == "__main__":
    rng = np.random.default_rng(0)
    fake = {
        "x": rng.standard_normal((16, 64, 64, 64), np.float32),
        "conv1_w": rng.standard_normal((64, 64, 3, 3), np.float32) * 0.05,
        "conv1_b": rng.standard_normal((64,), np.float32) * 0.05,
        "bn1_gamma": rng.uniform(0.5, 1.5, (64,)).astype(np.float32),
        "bn1_beta": rng.standard_normal((64,), np.float32) * 0.05,
        "bn1_mean": rng.standard_normal((64,), np.float32) * 0.05,
        "bn1_var": rng.uniform(0.5, 1.5, (64,)).astype(np.float32),
        "conv2_w": rng.standard_normal((96, 64, 3, 3), np.float32) * 0.05,
        "conv2_b": rng.standard_normal((96,), np.float32) * 0.05,
        "bn2_gamma": rng.uniform(0.5, 1.5, (96,)).astype(np.float32),
        "bn2_beta": rng.standard_normal((96,), np.float32) * 0.05,
        "bn2_mean": rng.standard_normal((96,), np.float32) * 0.05,
        "bn2_var": rng.uniform(0.5, 1.5, (96,)).astype(np.float32),
        "bases": rng.standard_normal((6, 9), np.float32),
        "coef": rng.standard_normal((128, 1024), np.float32) * 0.02,
    }
    o = kernel(**fake)
    print("out", o.shape, o.dtype)


# revision 19
# speedup vs baseline: 2.0050x; 1.1681x over previous
"""Trainium2 Bass kernel for nn_ADConv (adaptive-basis conv).

Math (per image, per pixel q=(h,w)):
  h1  = tanh(bn1(conv3x3(x)))                      # [64, H, W]
  bc  = tanh(bn2(conv3x3(h1)))                     # [96, H, W], channel = 6f+t
  PB[c,t,q]   = sum_k x[c, q+dk] * B[t,k]          # depthwise basis conv
  u[c,f,q]    = sum_t PB[c,t,q] * bc[6f+t, wq, hq] # per-pixel bilinear (DVE)
  out[o,w,h]  = sum_{c,f} coef[o, 16c+f] * u[c,f,q]

Sharding: data-parallel, batch 16 -> 2 images per NeuronCore, params
replicated. Everything computed in bf16 (fp32 PSUM accumulation).
"""

import os
import sys

import numpy as np

sys.path.insert(0, "/opt/trn_rl_repo")

import ml_dtypes

import concourse.bacc as bacc
import concourse.bass as bass
import concourse.mybir as mybir
import concourse.tile as tile
from concourse.ap import AP
from concourse.bass_utils import run_bass_kernel_spmd

BF16 = mybir.dt.bfloat16
F32 = mybir.dt.float32
AF = mybir.ActivationFunctionType
ALU = mybir.AluOpType

N_CORES = 8
IMGS = 2           # images per core
C = 64             # input channels
INTER = 64         # conv1 out channels
BCH = 96           # conv2 out channels = 16f * 6t
NT = 6             # TOTAL_BASES
NF = 16            # NUM_FA
O = 128            # output channels
H = W = 64
HP = 66            # padded spatial
Q = H * W          # 4096 pixels
RC = 8             # rows per chunk
NCHUNK = H // RC   # 8 chunks of 512 px
CH = RC * W        # 512 px per chunk
BN_EPS = 1e-5

_CACHE = {}


def _pbcast_src(tile_ap: AP, part_row: int, part_pitch: int, dims, offset_elems: int):
    """Manual AP: read from partition `part_row` of an SBUF tile, broadcast
    across 64 partitions (leading 0-stride dim), with free dims `dims`
    (list of [step, count]) starting at byte/elem offset `offset_elems`."""
    base = tile_ap  # AP covering whole tile, standard layout
    # Flat-element convention: partition p lives at p * part_pitch.
    off = part_row * part_pitch + offset_elems
    return AP(base.tensor, base.offset + off, [[0, 64]] + list(dims))


def build_graph():
    nc = bacc.Bacc(None, target_bir_lowering=False)

    xs = nc.declare_dram_parameter("xs", [IMGS, C, H, W], BF16, isOutput=False)
    w1p = nc.declare_dram_parameter("w1p", [128, 3, INTER], BF16, isOutput=False)
    w1s = nc.declare_dram_parameter("w1s", [C, 3, INTER], BF16, isOutput=False)
    s1 = nc.declare_dram_parameter("s1", [INTER, 1], F32, isOutput=False)
    b1 = nc.declare_dram_parameter("b1", [INTER, 1], F32, isOutput=False)
    w2p = nc.declare_dram_parameter("w2p", [128, 3, BCH], BF16, isOutput=False)
    w2s = nc.declare_dram_parameter("w2s", [INTER, 3, BCH], BF16, isOutput=False)
    s2 = nc.declare_dram_parameter("s2", [BCH, 1], F32, isOutput=False)
    b2 = nc.declare_dram_parameter("b2", [BCH, 1], F32, isOutput=False)
    wpbp = nc.declare_dram_parameter("wpbp", [128, 3, NT * C], BF16, isOutput=False)
    wpbs = nc.declare_dram_parameter("wpbs", [C, 3, NT * C], BF16, isOutput=False)
    coefp = nc.declare_dram_parameter("coefp", [128, NF, O], BF16, isOutput=False)
    out = nc.declare_dram_parameter("out", [IMGS, O, W, H], F32, isOutput=True)

    with tile.TileContext(nc) as tc:
        with (
            tc.tile_pool(name="persist", bufs=1) as pp,
            tc.tile_pool(name="bcrep", bufs=3) as bp,
            tc.tile_pool(name="uu", bufs=2) as up,
            tc.tile_pool(name="tmp", bufs=2) as tp,
            tc.tile_pool(name="osb", bufs=2) as op_,
            tc.tile_pool(name="dramb", bufs=1, space=bass.MemorySpace.DRAM) as dp,
            tc.tile_pool(name="ps_conv", bufs=1, space=bass.MemorySpace.PSUM) as pcv,
            tc.tile_pool(name="ps_pb", bufs=1, space=bass.MemorySpace.PSUM) as ppb,
            tc.tile_pool(name="ps_fin", bufs=1, space=bass.MemorySpace.PSUM) as pfn,
        ):
            # ---- persistent SBUF ----
            w1psb = pp.tile([128, 3, INTER], BF16, tag="w1psb")
            w1ssb = pp.tile([C, 3, INTER], BF16, tag="w1ssb")
            w2psb = pp.tile([128, 3, BCH], BF16, tag="w2psb")
            w2ssb = pp.tile([INTER, 3, BCH], BF16, tag="w2ssb")
            wpbpsb = pp.tile([128, 3, NT * C], BF16, tag="wpbpsb")
            wpbssb = pp.tile([C, 3, NT * C], BF16, tag="wpbssb")
            coefsb = pp.tile([128, NF, O], BF16, tag="coefsb")
            s1sb = pp.tile([INTER, 1], F32, tag="s1sb")
            b1sb = pp.tile([INTER, 1], F32, tag="b1sb")
            s2sb = pp.tile([BCH, 1], F32, tag="s2sb")
            b2sb = pp.tile([BCH, 1], F32, tag="b2sb")
            nc.sync.dma_start(w1psb[:], w1p[:])
            nc.sync.dma_start(w1ssb[:], w1s[:])
            nc.sync.dma_start(w2psb[:], w2p[:])
            nc.sync.dma_start(w2ssb[:], w2s[:])
            nc.sync.dma_start(wpbpsb[:], wpbp[:])
            nc.sync.dma_start(wpbssb[:], wpbs[:])
            nc.sync.dma_start(coefsb[:], coefp[:])
            nc.sync.dma_start(s1sb[:], s1[:])
            nc.sync.dma_start(b1sb[:], b1[:])
            nc.sync.dma_start(s2sb[:], s2[:])
            nc.sync.dma_start(b2sb[:], b2[:])

            xpad = []
            hpad = []
            bcs = []
            for i in range(IMGS):
                xp = pp.tile([128, HP, HP], BF16, tag=f"xpad{i}", name=f"xpad{i}")
                hp = pp.tile([128, HP, HP], BF16, tag=f"hpad{i}", name=f"hpad{i}")
                bi = pp.tile([BCH, H, W], BF16, tag=f"bc{i}", name=f"bc{i}")
                nc.vector.memset(xp[:], 0.0)
                nc.vector.memset(hp[:], 0.0)
                nc.sync.dma_start(xp[0:64, 1 : H + 1, 1 : W + 1], xs[i])
                nc.sync.dma_start(xp[64:128, 1 : H + 1, 0:W], xs[i])
                xpad.append(xp)
                hpad.append(hp)
                bcs.append(bi)
            pbt = [pp.tile([128, Q], BF16, tag=f"pbt{t}", name=f"pbt{t}") for t in range(NT)]
            bcd = [
                dp.tile([NCHUNK, BCH, CH], BF16, tag=f"bcd{i}", name=f"bcd{i}")
                for i in range(IMGS)
            ]

            # ---- stage 1+2+PB per image ----
            for i in range(IMGS):
                # conv1 -> bn -> tanh -> hpad
                for g in range(4):  # chunk groups of 2 (PSUM budget)
                    cps = pcv.tile([INTER, 2, RC, W], F32, tag="convps")
                    for m in range(6):
                        ki = m % 3
                        pair = m < 3
                        lhsT = w1psb[:, ki, :] if pair else w1ssb[:, ki, :]
                        for j in range(2):
                            h0 = (g * 2 + j) * RC
                            if pair:  # taps (ki,0)+(ki,1): hi half pre-shifted
                                rhs = xpad[i][:, h0 + ki : h0 + ki + RC, 0:W]
                            else:     # tap (ki,2)
                                rhs = xpad[i][0:64, h0 + ki : h0 + ki + RC, 2 : 2 + W]
                            nc.tensor.matmul(
                                cps[:, j], lhsT, rhs, start=(m == 0), stop=(m == 5)
                            )
                    for j in range(2):
                        h0 = (g * 2 + j) * RC
                        nc.scalar.activation(
                            hpad[i][0:64, h0 + 1 : h0 + 1 + RC, 1 : W + 1],
                            cps[:, j],
                            AF.Tanh,
                            bias=b1sb[:],
                            scale=s1sb[:],
                        )
                        nc.scalar.activation(
                            hpad[i][64:128, h0 + 1 : h0 + 1 + RC, 0:W],
                            cps[:, j],
                            AF.Tanh,
                            bias=b1sb[:],
                            scale=s1sb[:],
                        )
                # conv2 -> bn -> tanh -> bc
                for g in range(4):
                    cps = pcv.tile([BCH, 2, RC, W], F32, tag="convps")
                    for m in range(6):
                        ki = m % 3
                        pair = m < 3
                        lhsT = w2psb[:, ki, :] if pair else w2ssb[:, ki, :]
                        for j in range(2):
                            h0 = (g * 2 + j) * RC
                            if pair:
                                rhs = hpad[i][:, h0 + ki : h0 + ki + RC, 0:W]
                            else:
                                rhs = hpad[i][0:64, h0 + ki : h0 + ki + RC, 2 : 2 + W]
                            nc.tensor.matmul(
                                cps[:, j], lhsT, rhs, start=(m == 0), stop=(m == 5)
                            )
                    for j in range(2):
                        h0 = (g * 2 + j) * RC
                        # transposed store: bcT[ch, w_out, h_bc] so later
                        # per-pixel reads are contiguous
                        nc.scalar.activation(
                            bcs[i][:, :, h0 : h0 + RC].transpose([0, 2, 1]),
                            cps[:, j],
                            AF.Tanh,
                            bias=b2sb[:],
                            scale=s2sb[:],
                        )
            for i in range(IMGS):
                for ch in range(NCHUNK):
                    nc.sync.dma_start(
                        bcd[i][ch], bcs[i][:, ch * RC : (ch + 1) * RC, :].opt()
                    )

            # ---- per chunk: PB (PE) -> products (DVE) -> final w/ t-fold (PE) ----
            # u-chunks of 1024 px (2 PB chunks each)
            UCH = 2 * CH  # 1024
            for uc in range(NCHUNK // 2):
                for half in range(2):
                    ch = uc * 2 + half
                    h0 = ch * RC
                    c0 = h0 * W
                    for i in range(IMGS):
                        for j3 in range(3):
                            pps = ppb.tile(
                                [128, RC, W], F32, tag="pbps", name="pps", bufs=2
                            )
                            for m in range(6):
                                ki = m % 3
                                pair = m < 3
                                if pair:
                                    lhsT = wpbpsb[:, ki, j3 * 128 : (j3 + 1) * 128]
                                    rhs = xpad[i][:, h0 + ki : h0 + ki + RC, 0:W]
                                else:
                                    lhsT = wpbssb[:, ki, j3 * 128 : (j3 + 1) * 128]
                                    rhs = xpad[i][0:64, h0 + ki : h0 + ki + RC, 2 : 2 + W]
                                nc.tensor.matmul(
                                    pps[:], lhsT, rhs, start=(m == 0), stop=(m == 5)
                                )
                            nc.scalar.copy(
                                pbt[2 * j3][i * 64 : (i + 1) * 64, c0 : c0 + CH],
                                pps[0:64].opt(),
                            )
                            nc.scalar.copy(
                                pbt[2 * j3 + 1][i * 64 : (i + 1) * 64, c0 : c0 + CH],
                                pps[64:128].opt(),
                            )
                u0 = uc * UCH
                fps = [
                    pfn.tile([O, 2, CH], F32, tag=f"finps{i}", name=f"fps{i}", bufs=1)
                    for i in range(IMGS)
                ]
                for f in range(NF):
                    bcr = bp.tile([128, NT * UCH], BF16, tag="bcrv", name="bcrv", bufs=3)
                    bcr_full = bcr[:]
                    for i in range(IMGS):
                        dfull = bcd[i][:]
                        # two PB-chunks back to back in chunk-major bcd
                        for half in range(2):
                            srcap = AP(
                                dfull.tensor,
                                dfull.offset
                                + (uc * 2 + half) * BCH * CH
                                + f * NT * CH,
                                [[0, 64], [1, NT * CH]],
                            )
                            dstap = AP(
                                bcr_full.tensor,
                                bcr_full.offset
                                + i * 64 * (NT * UCH)
                                + half * CH,
                                [[NT * UCH, 64], [UCH, NT], [1, CH]],
                            )
                            iss = nc.sync if (f + i + half) % 2 == 0 else nc.scalar
                            iss.dma_start(dstap, srcap)
                    theta = tp.tile([128, NT * UCH], BF16, tag="theta", name="theta", bufs=3)
                    for t in range(NT):
                        nc.vector.tensor_mul(
                            theta[:, t * UCH : (t + 1) * UCH],
                            pbt[t][:, u0 : u0 + UCH],
                            bcr[:, t * UCH : (t + 1) * UCH],
                        )
                    # fold t-sum into final accumulation; img0/img1 row-packed
                    # into concurrent 64-row groups of the PE array
                    for t in range(NT):
                        for half in range(2):
                            for i in range(IMGS):
                                nc.tensor.matmul(
                                    fps[i][:, half],
                                    coefsb[i * 64 : (i + 1) * 64, f, :],
                                    theta[
                                        i * 64 : (i + 1) * 64,
                                        t * UCH + half * CH : t * UCH + (half + 1) * CH,
                                    ],
                                    start=(f == 0 and t == 0),
                                    stop=(f == NF - 1 and t == NT - 1),
                                    tile_position=(i * 64, 0),
                                )
                for i in range(IMGS):
                    for half in range(2):
                        ch = uc * 2 + half
                        h0 = ch * RC
                        osb = op_.tile([O, W, RC], F32, tag="osb", name="osb")
                        # store transposed: osb[o, w, h]
                        nc.scalar.copy(
                            osb[:].transpose([0, 2, 1]),
                            fps[i][:, half].opt(),
                        )
                        nc.sync.dma_start(out[i, :, :, h0 : h0 + RC], osb[:])

    nc.compile()
    return nc


def _prep_params(inputs):
    bf16 = ml_dtypes.bfloat16
    f32 = np.float32
    c1w = np.asarray(inputs["conv1_w"], f32)
    c2w = np.asarray(inputs["conv2_w"], f32)
    bases = np.asarray(inputs["bases"], f32)
    coef = np.asarray(inputs["coef"], f32)

    s1 = np.asarray(inputs["bn1_gamma"], f32) / np.sqrt(
        np.asarray(inputs["bn1_var"], f32) + BN_EPS
    )
    b1 = (np.asarray(inputs["conv1_b"], f32) - np.asarray(inputs["bn1_mean"], f32)) * s1 + np.asarray(
        inputs["bn1_beta"], f32
    )
    s2 = np.asarray(inputs["bn2_gamma"], f32) / np.sqrt(
        np.asarray(inputs["bn2_var"], f32) + BN_EPS
    )
    b2 = (np.asarray(inputs["conv2_b"], f32) - np.asarray(inputs["bn2_mean"], f32)) * s2 + np.asarray(
        inputs["bn2_beta"], f32
    )

    w1pk = np.zeros((128, 3, INTER), f32)
    w1sk = np.zeros((C, 3, INTER), f32)
    w2pk = np.zeros((128, 3, BCH), f32)
    w2sk = np.zeros((INTER, 3, BCH), f32)
    for ki in range(3):
        w1pk[0:64, ki] = c1w[:, :, ki, 0].T
        w1pk[64:128, ki] = c1w[:, :, ki, 1].T
        w1sk[:, ki] = c1w[:, :, ki, 2].T
        w2pk[0:64, ki] = c2w[:, :, ki, 0].T
        w2pk[64:128, ki] = c2w[:, :, ki, 1].T
        w2sk[:, ki] = c2w[:, :, ki, 2].T

    wpbpk = np.zeros((128, 3, NT * C), f32)
    wpbsk = np.zeros((C, 3, NT * C), f32)
    for t in range(NT):
        for c in range(C):
            for ki in range(3):
                wpbpk[c, ki, t * C + c] = bases[t, 3 * ki + 0]
                wpbpk[64 + c, ki, t * C + c] = bases[t, 3 * ki + 1]
                wpbsk[c, ki, t * C + c] = bases[t, 3 * ki + 2]

    cview = coef.reshape(O, C, NF)  # coef[o, 16c+f]
    coefp64 = np.ascontiguousarray(cview.transpose(1, 2, 0))  # [c, f, o]
    coefp = np.concatenate([coefp64, coefp64], axis=0)  # [128, f, o]

    return {
        "w1p": w1pk.astype(bf16),
        "w1s": w1sk.astype(bf16),
        "s1": s1.reshape(-1, 1).astype(f32),
        "b1": b1.reshape(-1, 1).astype(f32),
        "w2p": w2pk.astype(bf16),
        "w2s": w2sk.astype(bf16),
        "s2": s2.reshape(-1, 1).astype(f32),
        "b2": b2.reshape(-1, 1).astype(f32),
        "wpbp": wpbpk.astype(bf16),
        "wpbs": wpbsk.astype(bf16),
        "coefp": coefp.astype(bf16),
    }


def kernel(**inputs):
    if "nc" not in _CACHE:
        _CACHE["nc"] = build_graph()
    nc = _CACHE["nc"]

    params = _prep_params(inputs)
    x = np.asarray(inputs["x"], np.float32).astype(ml_dtypes.bfloat16)

    in_maps = []
    for core in range(N_CORES):
        m = dict(params)
        m["xs"] = np.ascontiguousarray(x[core * IMGS : (core + 1) * IMGS])
        in_maps.append(m)

    res = run_bass_kernel_spmd(nc, in_maps, core_ids=list(range(N_CORES)))
    outs = [r["out"] for r in res.results]
    return np.concatenate(outs, axis=0).astype(np.float32)


def _install_ntff_hook():
    """Shim antenv.axon_hooks with the trn_boot ctypes NTFF hook."""
    import types

    try:
        from antenv.axon_hooks import get_axon_ntff_profile_hook  # noqa
        return
    except ImportError:
        pass
    sys.path.insert(0, "/root/.axon_site/trn_agent_boot")
    import trn_boot

    hook = trn_boot._ntff_profile_via_ctypes("/opt/axon/libaxon_pjrt.so")
    mod_pkg = sys.modules.get("antenv")
    if mod_pkg is None:
        mod_pkg = types.ModuleType("antenv")
        sys.modules["antenv"] = mod_pkg
    mod = types.ModuleType("antenv.axon_hooks")
    mod.get_axon_ntff_profile_hook = lambda: hook
    mod.set_axon_ntff_profile_hook = lambda h: None
    sys.modules["antenv.axon_hooks"] = mod
    mod_pkg.axon_hooks = mod


def run_timed(inputs):
    """Run once with NTFF tracing; return exec_time_ns (or None)."""
    _install_ntff_hook()
    if "nc" not in _CACHE:
        _CACHE["nc"] = build_graph()
    nc = _CACHE["nc"]
    params = _prep_params(inputs)
    x = np.asarray(inputs["x"], np.float32).astype(ml_dtypes.bfloat16)
    in_maps = []
    for core in range(N_CORES):
        m = dict(params)
        m["xs"] = np.ascontiguousarray(x[core * IMGS : (core + 1) * IMGS])
        in_maps.append(m)
    res = run_bass_kernel_spmd(
        nc, in_maps, core_ids=list(range(N_CORES)), trace=True
    )
    print("trace profile_json:", res.profile_json)
    _CACHE["last_res"] = res
    return res.exec_time_ns


if __name__ == "__main__":
    rng = np.random.default_rng(0)
    fake = {
        "x": rng.standard_normal((16, 64, 64, 64), np.float32),
        "conv1_w": rng.standard_normal((64, 64, 3, 3), np.float32) * 0.05,
        "conv1_b": rng.standard_normal((64,), np.float32) * 0.05,
        "bn1_gamma": rng.uniform(0.5, 1.5, (64,)).astype(np.float32),
        "bn1_beta": rng.standard_normal((64,), np.float32) * 0.05,
        "bn1_mean": rng.standard_normal((64,), np.float32) * 0.05,
        "bn1_var": rng.uniform(0.5, 1.5, (64,)).astype(np.float32),
        "conv2_w": rng.standard_normal((96, 64, 3, 3), np.float32) * 0.05,
        "conv2_b": rng.standard_normal((96,), np.float32) * 0.05,
        "bn2_gamma": rng.uniform(0.5, 1.5, (96,)).astype(np.float32),
        "bn2_beta": rng.standard_normal((96,), np.float32) * 0.05,
        "bn2_mean": rng.standard_normal((96,), np.float32) * 0.05,
        "bn2_var": rng.uniform(0.5, 1.5, (96,)).astype(np.float32),
        "bases": rng.standard_normal((6, 9), np.float32),
        "coef": rng.standard_normal((128, 1024), np.float32) * 0.02,
    }
    o = kernel(**fake)
    print("out", o.shape, o.dtype)


# revision 20
# speedup vs baseline: 2.0416x; 1.0183x over previous
"""Trainium2 Bass kernel for nn_ADConv (adaptive-basis conv).

Math (per image, per pixel q=(h,w)):
  h1  = tanh(bn1(conv3x3(x)))                      # [64, H, W]
  bc  = tanh(bn2(conv3x3(h1)))                     # [96, H, W], channel = 6f+t
  PB[c,t,q]   = sum_k x[c, q+dk] * B[t,k]          # depthwise basis conv
  u[c,f,q]    = sum_t PB[c,t,q] * bc[6f+t, wq, hq] # per-pixel bilinear (DVE)
  out[o,w,h]  = sum_{c,f} coef[o, 16c+f] * u[c,f,q]

Sharding: data-parallel, batch 16 -> 2 images per NeuronCore, params
replicated. Everything computed in bf16 (fp32 PSUM accumulation).
"""

import os
import sys

import numpy as np

sys.path.insert(0, "/opt/trn_rl_repo")

import ml_dtypes

import concourse.bacc as bacc
import concourse.bass as bass
import concourse.mybir as mybir
import concourse.tile as tile
from concourse.ap import AP
from concourse.bass_utils import run_bass_kernel_spmd

BF16 = mybir.dt.bfloat16
F32 = mybir.dt.float32
AF = mybir.ActivationFunctionType
ALU = mybir.AluOpType

N_CORES = 8
IMGS = 2           # images per core
C = 64             # input channels
INTER = 64         # conv1 out channels
BCH = 96           # conv2 out channels = 16f * 6t
NT = 6             # TOTAL_BASES
NF = 16            # NUM_FA
O = 128            # output channels
H = W = 64
HP = 66            # padded spatial
Q = H * W          # 4096 pixels
RC = 8             # rows per chunk
NCHUNK = H // RC   # 8 chunks of 512 px
CH = RC * W        # 512 px per chunk
BN_EPS = 1e-5

_CACHE = {}


def _pbcast_src(tile_ap: AP, part_row: int, part_pitch: int, dims, offset_elems: int):
    """Manual AP: read from partition `part_row` of an SBUF tile, broadcast
    across 64 partitions (leading 0-stride dim), with free dims `dims`
    (list of [step, count]) starting at byte/elem offset `offset_elems`."""
    base = tile_ap  # AP covering whole tile, standard layout
    # Flat-element convention: partition p lives at p * part_pitch.
    off = part_row * part_pitch + offset_elems
    return AP(base.tensor, base.offset + off, [[0, 64]] + list(dims))


def build_graph():
    nc = bacc.Bacc(None, target_bir_lowering=False)

    xs = nc.declare_dram_parameter("xs", [IMGS, C, H, W], BF16, isOutput=False)
    w1p = nc.declare_dram_parameter("w1p", [128, 3, INTER], BF16, isOutput=False)
    w1s = nc.declare_dram_parameter("w1s", [C, 3, INTER], BF16, isOutput=False)
    s1 = nc.declare_dram_parameter("s1", [INTER, 1], F32, isOutput=False)
    b1 = nc.declare_dram_parameter("b1", [INTER, 1], F32, isOutput=False)
    w2p = nc.declare_dram_parameter("w2p", [128, 3, BCH], BF16, isOutput=False)
    w2s = nc.declare_dram_parameter("w2s", [INTER, 3, BCH], BF16, isOutput=False)
    s2 = nc.declare_dram_parameter("s2", [BCH, 1], F32, isOutput=False)
    b2 = nc.declare_dram_parameter("b2", [BCH, 1], F32, isOutput=False)
    wpbp = nc.declare_dram_parameter("wpbp", [128, 3, NT * C], BF16, isOutput=False)
    wpbs = nc.declare_dram_parameter("wpbs", [C, 3, NT * C], BF16, isOutput=False)
    coefp = nc.declare_dram_parameter("coefp", [128, NF, O], BF16, isOutput=False)
    out = nc.declare_dram_parameter("out", [IMGS, O, W, H], F32, isOutput=True)

    with tile.TileContext(nc) as tc:
        with (
            tc.tile_pool(name="persist", bufs=1) as pp,
            tc.tile_pool(name="bcrep", bufs=3) as bp,
            tc.tile_pool(name="uu", bufs=2) as up,
            tc.tile_pool(name="tmp", bufs=2) as tp,
            tc.tile_pool(name="osb", bufs=2) as op_,
            tc.tile_pool(name="dramb", bufs=1, space=bass.MemorySpace.DRAM) as dp,
            tc.tile_pool(name="ps_conv", bufs=1, space=bass.MemorySpace.PSUM) as pcv,
            tc.tile_pool(name="ps_pb", bufs=1, space=bass.MemorySpace.PSUM) as ppb,
            tc.tile_pool(name="ps_fin", bufs=1, space=bass.MemorySpace.PSUM) as pfn,
        ):
            # ---- persistent SBUF ----
            w1psb = pp.tile([128, 3, INTER], BF16, tag="w1psb")
            w1ssb = pp.tile([C, 3, INTER], BF16, tag="w1ssb")
            w2psb = pp.tile([128, 3, BCH], BF16, tag="w2psb")
            w2ssb = pp.tile([INTER, 3, BCH], BF16, tag="w2ssb")
            wpbpsb = pp.tile([128, 3, NT * C], BF16, tag="wpbpsb")
            wpbssb = pp.tile([C, 3, NT * C], BF16, tag="wpbssb")
            coefsb = pp.tile([128, NF, O], BF16, tag="coefsb")
            s1sb = pp.tile([INTER, 1], F32, tag="s1sb")
            b1sb = pp.tile([INTER, 1], F32, tag="b1sb")
            s2sb = pp.tile([BCH, 1], F32, tag="s2sb")
            b2sb = pp.tile([BCH, 1], F32, tag="b2sb")
            nc.sync.dma_start(w1psb[:], w1p[:])
            nc.sync.dma_start(w1ssb[:], w1s[:])
            nc.sync.dma_start(w2psb[:], w2p[:])
            nc.sync.dma_start(w2ssb[:], w2s[:])
            nc.sync.dma_start(wpbpsb[:], wpbp[:])
            nc.sync.dma_start(wpbssb[:], wpbs[:])
            nc.sync.dma_start(coefsb[:], coefp[:])
            nc.sync.dma_start(s1sb[:], s1[:])
            nc.sync.dma_start(b1sb[:], b1[:])
            nc.sync.dma_start(s2sb[:], s2[:])
            nc.sync.dma_start(b2sb[:], b2[:])

            xpad = []
            hpad = []
            bcs = []
            for i in range(IMGS):
                xp = pp.tile([128, HP, HP], BF16, tag=f"xpad{i}", name=f"xpad{i}")
                hp = pp.tile([128, HP, HP], BF16, tag=f"hpad{i}", name=f"hpad{i}")
                bi = pp.tile([BCH, H, W], BF16, tag=f"bc{i}", name=f"bc{i}")
                nc.vector.memset(xp[:], 0.0)
                nc.vector.memset(hp[:], 0.0)
                nc.sync.dma_start(xp[0:64, 1 : H + 1, 1 : W + 1], xs[i])
                nc.sync.dma_start(xp[64:128, 1 : H + 1, 0:W], xs[i])
                xpad.append(xp)
                hpad.append(hp)
                bcs.append(bi)
            pbt = [pp.tile([128, Q], BF16, tag=f"pbt{t}", name=f"pbt{t}") for t in range(NT)]
            bcd = [
                dp.tile([NCHUNK, BCH, CH], BF16, tag=f"bcd{i}", name=f"bcd{i}")
                for i in range(IMGS)
            ]

            # ---- stage 1+2+PB per image ----
            for i in range(IMGS):
                # conv1 -> bn -> tanh -> hpad
                for g in range(4):  # chunk groups of 2 (PSUM budget)
                    cps = pcv.tile([INTER, 2, RC, W], F32, tag="convps")
                    for m in range(6):
                        ki = m % 3
                        pair = m < 3
                        lhsT = w1psb[:, ki, :] if pair else w1ssb[:, ki, :]
                        for j in range(2):
                            h0 = (g * 2 + j) * RC
                            if pair:  # taps (ki,0)+(ki,1): hi half pre-shifted
                                rhs = xpad[i][:, h0 + ki : h0 + ki + RC, 0:W]
                            else:     # tap (ki,2)
                                rhs = xpad[i][0:64, h0 + ki : h0 + ki + RC, 2 : 2 + W]
                            nc.tensor.matmul(
                                cps[:, j], lhsT, rhs, start=(m == 0), stop=(m == 5)
                            )
                    for j in range(2):
                        h0 = (g * 2 + j) * RC
                        nc.scalar.activation(
                            hpad[i][0:64, h0 + 1 : h0 + 1 + RC, 1 : W + 1],
                            cps[:, j],
                            AF.Tanh,
                            bias=b1sb[:],
                            scale=s1sb[:],
                        )
                        nc.scalar.activation(
                            hpad[i][64:128, h0 + 1 : h0 + 1 + RC, 0:W],
                            cps[:, j],
                            AF.Tanh,
                            bias=b1sb[:],
                            scale=s1sb[:],
                        )
                # conv2 -> bn -> tanh -> bc
                for g in range(4):
                    cps = pcv.tile([BCH, 2, RC, W], F32, tag="convps")
                    for m in range(6):
                        ki = m % 3
                        pair = m < 3
                        lhsT = w2psb[:, ki, :] if pair else w2ssb[:, ki, :]
                        for j in range(2):
                            h0 = (g * 2 + j) * RC
                            if pair:
                                rhs = hpad[i][:, h0 + ki : h0 + ki + RC, 0:W]
                            else:
                                rhs = hpad[i][0:64, h0 + ki : h0 + ki + RC, 2 : 2 + W]
                            nc.tensor.matmul(
                                cps[:, j], lhsT, rhs, start=(m == 0), stop=(m == 5)
                            )
                    for j in range(2):
                        h0 = (g * 2 + j) * RC
                        # transposed store: bcT[ch, w_out, h_bc] so later
                        # per-pixel reads are contiguous
                        nc.scalar.activation(
                            bcs[i][:, :, h0 : h0 + RC].transpose([0, 2, 1]),
                            cps[:, j],
                            AF.Tanh,
                            bias=b2sb[:],
                            scale=s2sb[:],
                        )
            for i in range(IMGS):
                for ch in range(NCHUNK):
                    nc.sync.dma_start(
                        bcd[i][ch], bcs[i][:, ch * RC : (ch + 1) * RC, :].opt()
                    )

            # ---- per chunk: PB (PE) -> products (DVE) -> final w/ t-fold (PE) ----
            # u-chunks of 1024 px (2 PB chunks each)
            UCH = 2 * CH  # 1024
            for uc in range(NCHUNK // 2):
                for half in range(2):
                    ch = uc * 2 + half
                    h0 = ch * RC
                    c0 = h0 * W
                    for i in range(IMGS):
                        for j3 in range(3):
                            pps = ppb.tile(
                                [128, RC, W], F32, tag="pbps", name="pps", bufs=2
                            )
                            for m in range(6):
                                ki = m % 3
                                pair = m < 3
                                if pair:
                                    lhsT = wpbpsb[:, ki, j3 * 128 : (j3 + 1) * 128]
                                    rhs = xpad[i][:, h0 + ki : h0 + ki + RC, 0:W]
                                else:
                                    lhsT = wpbssb[:, ki, j3 * 128 : (j3 + 1) * 128]
                                    rhs = xpad[i][0:64, h0 + ki : h0 + ki + RC, 2 : 2 + W]
                                nc.tensor.matmul(
                                    pps[:], lhsT, rhs, start=(m == 0), stop=(m == 5)
                                )
                            nc.vector.tensor_copy(
                                pbt[2 * j3][i * 64 : (i + 1) * 64, c0 : c0 + CH],
                                pps[0:64].opt(),
                            )
                            nc.vector.tensor_copy(
                                pbt[2 * j3 + 1][i * 64 : (i + 1) * 64, c0 : c0 + CH],
                                pps[64:128].opt(),
                            )
                u0 = uc * UCH
                fps = [
                    pfn.tile([O, 2, CH], F32, tag=f"finps{i}", name=f"fps{i}", bufs=1)
                    for i in range(IMGS)
                ]
                for f in range(NF):
                    bcr = bp.tile([128, NT * UCH], BF16, tag="bcrv", name="bcrv", bufs=3)
                    bcr_full = bcr[:]
                    for i in range(IMGS):
                        dfull = bcd[i][:]
                        # two PB-chunks back to back in chunk-major bcd
                        for half in range(2):
                            srcap = AP(
                                dfull.tensor,
                                dfull.offset
                                + (uc * 2 + half) * BCH * CH
                                + f * NT * CH,
                                [[0, 64], [1, NT * CH]],
                            )
                            dstap = AP(
                                bcr_full.tensor,
                                bcr_full.offset
                                + i * 64 * (NT * UCH)
                                + half * CH,
                                [[NT * UCH, 64], [UCH, NT], [1, CH]],
                            )
                            iss = (nc.sync, nc.scalar, nc.gpsimd)[(f + 2 * i + half) % 3]
                            iss.dma_start(dstap, srcap)
                    theta = tp.tile([128, NT * UCH], BF16, tag="theta", name="theta", bufs=3)
                    for t in range(NT):
                        nc.vector.tensor_mul(
                            theta[:, t * UCH : (t + 1) * UCH],
                            pbt[t][:, u0 : u0 + UCH],
                            bcr[:, t * UCH : (t + 1) * UCH],
                        )
                    # fold t-sum into final accumulation; img0/img1 row-packed
                    # into concurrent 64-row groups of the PE array
                    for t in range(NT):
                        for half in range(2):
                            for i in range(IMGS):
                                nc.tensor.matmul(
                                    fps[i][:, half],
                                    coefsb[i * 64 : (i + 1) * 64, f, :],
                                    theta[
                                        i * 64 : (i + 1) * 64,
                                        t * UCH + half * CH : t * UCH + (half + 1) * CH,
                                    ],
                                    start=(f == 0 and t == 0),
                                    stop=(f == NF - 1 and t == NT - 1),
                                    tile_position=(i * 64, 0),
                                )
                for i in range(IMGS):
                    for half in range(2):
                        ch = uc * 2 + half
                        h0 = ch * RC
                        osb = op_.tile([O, W, RC], F32, tag="osb", name="osb")
                        # store transposed: osb[o, w, h]
                        nc.scalar.copy(
                            osb[:].transpose([0, 2, 1]),
                            fps[i][:, half].opt(),
                        )
                        nc.sync.dma_start(out[i, :, :, h0 : h0 + RC], osb[:])

    nc.compile()
    return nc


def _prep_params(inputs):
    bf16 = ml_dtypes.bfloat16
    f32 = np.float32
    c1w = np.asarray(inputs["conv1_w"], f32)
    c2w = np.asarray(inputs["conv2_w"], f32)
    bases = np.asarray(inputs["bases"], f32)
    coef = np.asarray(inputs["coef"], f32)

    s1 = np.asarray(inputs["bn1_gamma"], f32) / np.sqrt(
        np.asarray(inputs["bn1_var"], f32) + BN_EPS
    )
    b1 = (np.asarray(inputs["conv1_b"], f32) - np.asarray(inputs["bn1_mean"], f32)) * s1 + np.asarray(
        inputs["bn1_beta"], f32
    )
    s2 = np.asarray(inputs["bn2_gamma"], f32) / np.sqrt(
        np.asarray(inputs["bn2_var"], f32) + BN_EPS
    )
    b2 = (np.asarray(inputs["conv2_b"], f32) - np.asarray(inputs["bn2_mean"], f32)) * s2 + np.asarray(
        inputs["bn2_beta"], f32
    )

    w1pk = np.zeros((128, 3, INTER), f32)
    w1sk = np.zeros((C, 3, INTER), f32)
    w2pk = np.zeros((128, 3, BCH), f32)
    w2sk = np.zeros((INTER, 3, BCH), f32)
    for ki in range(3):
        w1pk[0:64, ki] = c1w[:, :, ki, 0].T
        w1pk[64:128, ki] = c1w[:, :, ki, 1].T
        w1sk[:, ki] = c1w[:, :, ki, 2].T
        w2pk[0:64, ki] = c2w[:, :, ki, 0].T
        w2pk[64:128, ki] = c2w[:, :, ki, 1].T
        w2sk[:, ki] = c2w[:, :, ki, 2].T

    wpbpk = np.zeros((128, 3, NT * C), f32)
    wpbsk = np.zeros((C, 3, NT * C), f32)
    for t in range(NT):
        for c in range(C):
            for ki in range(3):
                wpbpk[c, ki, t * C + c] = bases[t, 3 * ki + 0]
                wpbpk[64 + c, ki, t * C + c] = bases[t, 3 * ki + 1]
                wpbsk[c, ki, t * C + c] = bases[t, 3 * ki + 2]

    cview = coef.reshape(O, C, NF)  # coef[o, 16c+f]
    coefp64 = np.ascontiguousarray(cview.transpose(1, 2, 0))  # [c, f, o]
    coefp = np.concatenate([coefp64, coefp64], axis=0)  # [128, f, o]

    return {
        "w1p": w1pk.astype(bf16),
        "w1s": w1sk.astype(bf16),
        "s1": s1.reshape(-1, 1).astype(f32),
        "b1": b1.reshape(-1, 1).astype(f32),
        "w2p": w2pk.astype(bf16),
        "w2s": w2sk.astype(bf16),
        "s2": s2.reshape(-1, 1).astype(f32),
        "b2": b2.reshape(-1, 1).astype(f32),
        "wpbp": wpbpk.astype(bf16),
        "wpbs": wpbsk.astype(bf16),
        "coefp": coefp.astype(bf16),
    }


def kernel(**inputs):
    if "nc" not in _CACHE:
        _CACHE["nc"] = build_graph()
    nc = _CACHE["nc"]

    params = _prep_params(inputs)
    x = np.asarray(inputs["x"], np.float32).astype(ml_dtypes.bfloat16)

    in_maps = []
    for core in range(N_CORES):
        m = dict(params)
        m["xs"] = np.ascontiguousarray(x[core * IMGS : (core + 1) * IMGS])
        in_maps.append(m)

    res = run_bass_kernel_spmd(nc, in_maps, core_ids=list(range(N_CORES)))
    outs = [r["out"] for r in res.results]
    return np.concatenate(outs, axis=0).astype(np.float32)


def _install_ntff_hook():
    """Shim antenv.axon_hooks with the trn_boot ctypes NTFF hook."""
    import types

    try:
        from antenv.axon_hooks import get_axon_ntff_profile_hook  # noqa
        return
    except ImportError:
        pass
    sys.path.insert(0, "/root/.axon_site/trn_agent_boot")
    import trn_boot

    hook = trn_boot._ntff_profile_via_ctypes("/opt/axon/libaxon_pjrt.so")
    mod_pkg = sys.modules.get("antenv")
    if mod_pkg is None:
        mod_pkg = types.ModuleType("antenv")
        sys.modules["antenv"] = mod_pkg
    mod = types.ModuleType("antenv.axon_hooks")
    mod.get_axon_ntff_profile_hook = lambda: hook
    mod.set_axon_ntff_profile_hook = lambda h: None
    sys.modules["antenv.axon_hooks"] = mod
    mod_pkg.axon_hooks = mod


def run_timed(inputs):
    """Run once with NTFF tracing; return exec_time_ns (or None)."""
    _install_ntff_hook()
    if "nc" not in _CACHE:
        _CACHE["nc"] = build_graph()
    nc = _CACHE["nc"]
    params = _prep_params(inputs)
    x = np.asarray(inputs["x"], np.float32).astype(ml_dtypes.bfloat16)
    in_maps = []
    for core in range(N_CORES):
        m = dict(params)
        m["xs"] = np.ascontiguousarray(x[core * IMGS : (core + 1) * IMGS])
        in_maps.append(m)
    res = run_bass_kernel_spmd(
        nc, in_maps, core_ids=list(range(N_CORES)), trace=True
    )
    print("trace profile_json:", res.profile_json)
    _CACHE["last_res"] = res
    return res.exec_time_ns


if __name__ == "__main__":
    rng = np.random.default_rng(0)
    fake = {
        "x": rng.standard_normal((16, 64, 64, 64), np.float32),
        "conv1_w": rng.standard_normal((64, 64, 3, 3), np.float32) * 0.05,
        "conv1_b": rng.standard_normal((64,), np.float32) * 0.05,
        "bn1_gamma": rng.uniform(0.5, 1.5, (64,)).astype(np.float32),
        "bn1_beta": rng.standard_normal((64,), np.float32) * 0.05,
        "bn1_mean": rng.standard_normal((64,), np.float32) * 0.05,
        "bn1_var": rng.uniform(0.5, 1.5, (64,)).astype(np.float32),
        "conv2_w": rng.standard_normal((96, 64, 3, 3), np.float32) * 0.05,
        "conv2_b": rng.standard_normal((96,), np.float32) * 0.05,
        "bn2_gamma": rng.uniform(0.5, 1.5, (96,)).astype(np.float32),
        "bn2_beta": rng.standard_normal((96,), np.float32) * 0.05,
        "bn2_mean": rng.standard_normal((96,), np.float32) * 0.05,
        "bn2_var": rng.uniform(0.5, 1.5, (96,)).astype(np.float32),
        "bases": rng.standard_normal((6, 9), np.float32),
        "coef": rng.standard_normal((128, 1024), np.float32) * 0.02,
    }
    o = kernel(**fake)
    print("out", o.shape, o.dtype)


# revision 21
# speedup vs baseline: 2.1360x; 1.0463x over previous
"""Trainium2 Bass kernel for nn_ADConv (adaptive-basis conv).

Math (per image, per pixel q=(h,w)):
  h1  = tanh(bn1(conv3x3(x)))                      # [64, H, W]
  bc  = tanh(bn2(conv3x3(h1)))                     # [96, H, W], channel = 6f+t
  PB[c,t,q]   = sum_k x[c, q+dk] * B[t,k]          # depthwise basis conv
  u[c,f,q]    = sum_t PB[c,t,q] * bc[6f+t, wq, hq] # per-pixel bilinear (DVE)
  out[o,w,h]  = sum_{c,f} coef[o, 16c+f] * u[c,f,q]

Sharding: data-parallel, batch 16 -> 2 images per NeuronCore, params
replicated. Everything computed in bf16 (fp32 PSUM accumulation).
"""

import os
import sys

import numpy as np

sys.path.insert(0, "/opt/trn_rl_repo")

import ml_dtypes

import concourse.bacc as bacc
import concourse.bass as bass
import concourse.mybir as mybir
import concourse.tile as tile
from concourse.ap import AP
from concourse.bass_utils import run_bass_kernel_spmd

BF16 = mybir.dt.bfloat16
F32 = mybir.dt.float32
AF = mybir.ActivationFunctionType
ALU = mybir.AluOpType

N_CORES = 8
IMGS = 2           # images per core
C = 64             # input channels
INTER = 64         # conv1 out channels
BCH = 96           # conv2 out channels = 16f * 6t
NT = 6             # TOTAL_BASES
NF = 16            # NUM_FA
O = 128            # output channels
H = W = 64
HP = 66            # padded spatial
Q = H * W          # 4096 pixels
RC = 8             # rows per chunk
NCHUNK = H // RC   # 8 chunks of 512 px
CH = RC * W        # 512 px per chunk
BN_EPS = 1e-5

_CACHE = {}


def _pbcast_src(tile_ap: AP, part_row: int, part_pitch: int, dims, offset_elems: int):
    """Manual AP: read from partition `part_row` of an SBUF tile, broadcast
    across 64 partitions (leading 0-stride dim), with free dims `dims`
    (list of [step, count]) starting at byte/elem offset `offset_elems`."""
    base = tile_ap  # AP covering whole tile, standard layout
    # Flat-element convention: partition p lives at p * part_pitch.
    off = part_row * part_pitch + offset_elems
    return AP(base.tensor, base.offset + off, [[0, 64]] + list(dims))


def build_graph():
    nc = bacc.Bacc(None, target_bir_lowering=False)

    xs = nc.declare_dram_parameter("xs", [IMGS, C, H, W], BF16, isOutput=False)
    w1p = nc.declare_dram_parameter("w1p", [128, 3, INTER], BF16, isOutput=False)
    w1s = nc.declare_dram_parameter("w1s", [C, 3, INTER], BF16, isOutput=False)
    s1 = nc.declare_dram_parameter("s1", [INTER, 1], F32, isOutput=False)
    b1 = nc.declare_dram_parameter("b1", [INTER, 1], F32, isOutput=False)
    w2p = nc.declare_dram_parameter("w2p", [128, 3, BCH], BF16, isOutput=False)
    w2s = nc.declare_dram_parameter("w2s", [INTER, 3, BCH], BF16, isOutput=False)
    s2 = nc.declare_dram_parameter("s2", [BCH, 1], F32, isOutput=False)
    b2 = nc.declare_dram_parameter("b2", [BCH, 1], F32, isOutput=False)
    wpbp = nc.declare_dram_parameter("wpbp", [128, 3, NT * C], BF16, isOutput=False)
    wpbs = nc.declare_dram_parameter("wpbs", [C, 3, NT * C], BF16, isOutput=False)
    coefp = nc.declare_dram_parameter("coefp", [128, NF, O], BF16, isOutput=False)
    out = nc.declare_dram_parameter("out", [IMGS, O, W, H], F32, isOutput=True)

    with tile.TileContext(nc) as tc:
        with (
            tc.tile_pool(name="persist", bufs=1) as pp,
            tc.tile_pool(name="bcrep", bufs=3) as bp,
            tc.tile_pool(name="uu", bufs=2) as up,
            tc.tile_pool(name="tmp", bufs=2) as tp,
            tc.tile_pool(name="osb", bufs=2) as op_,
            tc.tile_pool(name="dramb", bufs=1, space=bass.MemorySpace.DRAM) as dp,
            tc.tile_pool(name="ps_conv", bufs=1, space=bass.MemorySpace.PSUM) as pcv,
            tc.tile_pool(name="ps_pb", bufs=1, space=bass.MemorySpace.PSUM) as ppb,
            tc.tile_pool(name="ps_fin", bufs=1, space=bass.MemorySpace.PSUM) as pfn,
        ):
            # ---- persistent SBUF ----
            w1psb = pp.tile([128, 3, INTER], BF16, tag="w1psb")
            w1ssb = pp.tile([C, 3, INTER], BF16, tag="w1ssb")
            w2psb = pp.tile([128, 3, BCH], BF16, tag="w2psb")
            w2ssb = pp.tile([INTER, 3, BCH], BF16, tag="w2ssb")
            wpbpsb = pp.tile([128, 3, NT * C], BF16, tag="wpbpsb")
            wpbssb = pp.tile([C, 3, NT * C], BF16, tag="wpbssb")
            coefsb = pp.tile([128, NF, O], BF16, tag="coefsb")
            s1sb = pp.tile([INTER, 1], F32, tag="s1sb")
            b1sb = pp.tile([INTER, 1], F32, tag="b1sb")
            s2sb = pp.tile([BCH, 1], F32, tag="s2sb")
            b2sb = pp.tile([BCH, 1], F32, tag="b2sb")
            nc.sync.dma_start(w1psb[:], w1p[:])
            nc.sync.dma_start(w1ssb[:], w1s[:])
            nc.sync.dma_start(w2psb[:], w2p[:])
            nc.sync.dma_start(w2ssb[:], w2s[:])
            nc.sync.dma_start(wpbpsb[:], wpbp[:])
            nc.sync.dma_start(wpbssb[:], wpbs[:])
            nc.sync.dma_start(coefsb[:], coefp[:])
            nc.sync.dma_start(s1sb[:], s1[:])
            nc.sync.dma_start(b1sb[:], b1[:])
            nc.sync.dma_start(s2sb[:], s2[:])
            nc.sync.dma_start(b2sb[:], b2[:])

            xpad = []
            hpad = []
            bcs = []
            for i in range(IMGS):
                xp = pp.tile([128, HP, HP], BF16, tag=f"xpad{i}", name=f"xpad{i}")
                hp = pp.tile([128, HP, HP], BF16, tag=f"hpad{i}", name=f"hpad{i}")
                bi = pp.tile([BCH, H, W], BF16, tag=f"bc{i}", name=f"bc{i}")
                nc.vector.memset(xp[:], 0.0)
                nc.vector.memset(hp[:], 0.0)
                nc.sync.dma_start(xp[0:64, 1 : H + 1, 1 : W + 1], xs[i])
                nc.sync.dma_start(xp[64:128, 1 : H + 1, 0:W], xs[i])
                xpad.append(xp)
                hpad.append(hp)
                bcs.append(bi)
            pbt2 = [
                [
                    pp.tile([128, Q], BF16, tag=f"pbt{i}_{j3}", name=f"pbt{i}_{j3}")
                    for j3 in range(3)
                ]
                for i in range(IMGS)
            ]
            bcd = [
                dp.tile([NCHUNK, BCH, CH], BF16, tag=f"bcd{i}", name=f"bcd{i}")
                for i in range(IMGS)
            ]

            # ---- stage 1+2+PB per image ----
            for i in range(IMGS):
                # conv1 -> bn -> tanh -> hpad
                for g in range(4):  # chunk groups of 2 (PSUM budget)
                    cps = pcv.tile([INTER, 2, RC, W], F32, tag="convps")
                    for m in range(6):
                        ki = m % 3
                        pair = m < 3
                        lhsT = w1psb[:, ki, :] if pair else w1ssb[:, ki, :]
                        for j in range(2):
                            h0 = (g * 2 + j) * RC
                            if pair:  # taps (ki,0)+(ki,1): hi half pre-shifted
                                rhs = xpad[i][:, h0 + ki : h0 + ki + RC, 0:W]
                            else:     # tap (ki,2)
                                rhs = xpad[i][0:64, h0 + ki : h0 + ki + RC, 2 : 2 + W]
                            nc.tensor.matmul(
                                cps[:, j], lhsT, rhs, start=(m == 0), stop=(m == 5)
                            )
                    for j in range(2):
                        h0 = (g * 2 + j) * RC
                        nc.scalar.activation(
                            hpad[i][0:64, h0 + 1 : h0 + 1 + RC, 1 : W + 1],
                            cps[:, j],
                            AF.Tanh,
                            bias=b1sb[:],
                            scale=s1sb[:],
                        )
                        nc.scalar.activation(
                            hpad[i][64:128, h0 + 1 : h0 + 1 + RC, 0:W],
                            cps[:, j],
                            AF.Tanh,
                            bias=b1sb[:],
                            scale=s1sb[:],
                        )
                # conv2 -> bn -> tanh -> bc
                for g in range(4):
                    cps = pcv.tile([BCH, 2, RC, W], F32, tag="convps")
                    for m in range(6):
                        ki = m % 3
                        pair = m < 3
                        lhsT = w2psb[:, ki, :] if pair else w2ssb[:, ki, :]
                        for j in range(2):
                            h0 = (g * 2 + j) * RC
                            if pair:
                                rhs = hpad[i][:, h0 + ki : h0 + ki + RC, 0:W]
                            else:
                                rhs = hpad[i][0:64, h0 + ki : h0 + ki + RC, 2 : 2 + W]
                            nc.tensor.matmul(
                                cps[:, j], lhsT, rhs, start=(m == 0), stop=(m == 5)
                            )
                    for j in range(2):
                        h0 = (g * 2 + j) * RC
                        # transposed store: bcT[ch, w_out, h_bc] so later
                        # per-pixel reads are contiguous
                        nc.scalar.activation(
                            bcs[i][:, :, h0 : h0 + RC].transpose([0, 2, 1]),
                            cps[:, j],
                            AF.Tanh,
                            bias=b2sb[:],
                            scale=s2sb[:],
                        )
            for i in range(IMGS):
                for ch in range(NCHUNK):
                    nc.sync.dma_start(
                        bcd[i][ch], bcs[i][:, ch * RC : (ch + 1) * RC, :].opt()
                    )

            # ---- per chunk: PB (PE) -> products (DVE) -> final w/ t-fold (PE) ----
            UCH = 2 * CH  # 1024-px u-chunks
            for uc in range(NCHUNK // 2):
                for half in range(2):
                    ch = uc * 2 + half
                    h0 = ch * RC
                    c0 = h0 * W
                    for i in range(IMGS):
                        for j3 in range(3):
                            pps = ppb.tile(
                                [128, RC, W], F32, tag="pbps", name="pps", bufs=2
                            )
                            for m in range(6):
                                ki = m % 3
                                pair = m < 3
                                if pair:
                                    lhsT = wpbpsb[:, ki, j3 * 128 : (j3 + 1) * 128]
                                    rhs = xpad[i][:, h0 + ki : h0 + ki + RC, 0:W]
                                else:
                                    lhsT = wpbssb[:, ki, j3 * 128 : (j3 + 1) * 128]
                                    rhs = xpad[i][0:64, h0 + ki : h0 + ki + RC, 2 : 2 + W]
                                nc.tensor.matmul(
                                    pps[:], lhsT, rhs, start=(m == 0), stop=(m == 5)
                                )
                            nc.vector.tensor_copy(
                                pbt2[i][j3][:, c0 : c0 + CH], pps[:].opt()
                            )
                u0 = uc * UCH
                fps = [
                    pfn.tile([O, 2, CH], F32, tag=f"finps{i}", name=f"fps{i}", bufs=1)
                    for i in range(IMGS)
                ]
                for f in range(NF):
                    for i in range(IMGS):
                        # bcr_pair[p, j3, half, q]: p<64 <- bc row (f*6+2*j3),
                        # p>=64 <- bc row (f*6+2*j3+1), replicated across 64
                        bcr = bp.tile(
                            [128, 3 * UCH], BF16, tag="bcrv", name="bcrv", bufs=4
                        )
                        bcr_full = bcr[:]
                        dfull = bcd[i][:]
                        for ph in range(2):
                            for half in range(2):
                                srcap = AP(
                                    dfull.tensor,
                                    dfull.offset
                                    + (uc * 2 + half) * BCH * CH
                                    + (f * NT + ph) * CH,
                                    [[0, 64], [2 * CH, 3], [1, CH]],
                                )
                                dstap = AP(
                                    bcr_full.tensor,
                                    bcr_full.offset
                                    + ph * 64 * (3 * UCH)
                                    + half * CH,
                                    [[3 * UCH, 64], [UCH, 3], [1, CH]],
                                )
                                iss = (nc.sync, nc.scalar, nc.gpsimd)[
                                    (f + 2 * i + 2 * ph + half) % 3
                                ]
                                iss.dma_start(dstap, srcap)
                        theta = tp.tile(
                            [128, 3 * UCH], BF16, tag="theta", name="theta", bufs=4
                        )
                        for j3 in range(3):
                            nc.vector.tensor_mul(
                                theta[:, j3 * UCH : (j3 + 1) * UCH],
                                pbt2[i][j3][:, u0 : u0 + UCH],
                                bcr[:, j3 * UCH : (j3 + 1) * UCH],
                            )
                        for j3 in range(3):
                            for half in range(2):
                                nc.tensor.matmul(
                                    fps[i][:, half],
                                    coefsb[:, f, :],
                                    theta[
                                        :,
                                        j3 * UCH + half * CH : j3 * UCH + (half + 1) * CH,
                                    ],
                                    start=(f == 0 and j3 == 0),
                                    stop=(f == NF - 1 and j3 == 2),
                                )
                for i in range(IMGS):
                    for half in range(2):
                        ch = uc * 2 + half
                        h0 = ch * RC
                        osb = op_.tile([O, W, RC], F32, tag="osb", name="osb")
                        nc.scalar.copy(
                            osb[:].transpose([0, 2, 1]),
                            fps[i][:, half].opt(),
                        )
                        nc.sync.dma_start(out[i, :, :, h0 : h0 + RC], osb[:])

    nc.compile()
    return nc


def _prep_params(inputs):
    bf16 = ml_dtypes.bfloat16
    f32 = np.float32
    c1w = np.asarray(inputs["conv1_w"], f32)
    c2w = np.asarray(inputs["conv2_w"], f32)
    bases = np.asarray(inputs["bases"], f32)
    coef = np.asarray(inputs["coef"], f32)

    s1 = np.asarray(inputs["bn1_gamma"], f32) / np.sqrt(
        np.asarray(inputs["bn1_var"], f32) + BN_EPS
    )
    b1 = (np.asarray(inputs["conv1_b"], f32) - np.asarray(inputs["bn1_mean"], f32)) * s1 + np.asarray(
        inputs["bn1_beta"], f32
    )
    s2 = np.asarray(inputs["bn2_gamma"], f32) / np.sqrt(
        np.asarray(inputs["bn2_var"], f32) + BN_EPS
    )
    b2 = (np.asarray(inputs["conv2_b"], f32) - np.asarray(inputs["bn2_mean"], f32)) * s2 + np.asarray(
        inputs["bn2_beta"], f32
    )

    w1pk = np.zeros((128, 3, INTER), f32)
    w1sk = np.zeros((C, 3, INTER), f32)
    w2pk = np.zeros((128, 3, BCH), f32)
    w2sk = np.zeros((INTER, 3, BCH), f32)
    for ki in range(3):
        w1pk[0:64, ki] = c1w[:, :, ki, 0].T
        w1pk[64:128, ki] = c1w[:, :, ki, 1].T
        w1sk[:, ki] = c1w[:, :, ki, 2].T
        w2pk[0:64, ki] = c2w[:, :, ki, 0].T
        w2pk[64:128, ki] = c2w[:, :, ki, 1].T
        w2sk[:, ki] = c2w[:, :, ki, 2].T

    wpbpk = np.zeros((128, 3, NT * C), f32)
    wpbsk = np.zeros((C, 3, NT * C), f32)
    for t in range(NT):
        for c in range(C):
            for ki in range(3):
                wpbpk[c, ki, t * C + c] = bases[t, 3 * ki + 0]
                wpbpk[64 + c, ki, t * C + c] = bases[t, 3 * ki + 1]
                wpbsk[c, ki, t * C + c] = bases[t, 3 * ki + 2]

    cview = coef.reshape(O, C, NF)  # coef[o, 16c+f]
    coefp64 = np.ascontiguousarray(cview.transpose(1, 2, 0))  # [c, f, o]
    coefp = np.concatenate([coefp64, coefp64], axis=0)  # [128, f, o]

    return {
        "w1p": w1pk.astype(bf16),
        "w1s": w1sk.astype(bf16),
        "s1": s1.reshape(-1, 1).astype(f32),
        "b1": b1.reshape(-1, 1).astype(f32),
        "w2p": w2pk.astype(bf16),
        "w2s": w2sk.astype(bf16),
        "s2": s2.reshape(-1, 1).astype(f32),
        "b2": b2.reshape(-1, 1).astype(f32),
        "wpbp": wpbpk.astype(bf16),
        "wpbs": wpbsk.astype(bf16),
        "coefp": coefp.astype(bf16),
    }


def kernel(**inputs):
    if "nc" not in _CACHE:
        _CACHE["nc"] = build_graph()
    nc = _CACHE["nc"]

    params = _prep_params(inputs)
    x = np.asarray(inputs["x"], np.float32).astype(ml_dtypes.bfloat16)

    in_maps = []
    for core in range(N_CORES):
        m = dict(params)
        m["xs"] = np.ascontiguousarray(x[core * IMGS : (core + 1) * IMGS])
        in_maps.append(m)

    res = run_bass_kernel_spmd(nc, in_maps, core_ids=list(range(N_CORES)))
    outs = [r["out"] for r in res.results]
    return np.concatenate(outs, axis=0).astype(np.float32)


def _install_ntff_hook():
    """Shim antenv.axon_hooks with the trn_boot ctypes NTFF hook."""
    import types

    try:
        from antenv.axon_hooks import get_axon_ntff_profile_hook  # noqa
        return
    except ImportError:
        pass
    sys.path.insert(0, "/root/.axon_site/trn_agent_boot")
    import trn_boot

    hook = trn_boot._ntff_profile_via_ctypes("/opt/axon/libaxon_pjrt.so")
    mod_pkg = sys.modules.get("antenv")
    if mod_pkg is None:
        mod_pkg = types.ModuleType("antenv")
        sys.modules["antenv"] = mod_pkg
    mod = types.ModuleType("antenv.axon_hooks")
    mod.get_axon_ntff_profile_hook = lambda: hook
    mod.set_axon_ntff_profile_hook = lambda h: None
    sys.modules["antenv.axon_hooks"] = mod
    mod_pkg.axon_hooks = mod


def run_timed(inputs):
    """Run once with NTFF tracing; return exec_time_ns (or None)."""
    _install_ntff_hook()
    if "nc" not in _CACHE:
        _CACHE["nc"] = build_graph()
    nc = _CACHE["nc"]
    params = _prep_params(inputs)
    x = np.asarray(inputs["x"], np.float32).astype(ml_dtypes.bfloat16)
    in_maps = []
    for core in range(N_CORES):
        m = dict(params)
        m["xs"] = np.ascontiguousarray(x[core * IMGS : (core + 1) * IMGS])
        in_maps.append(m)
    res = run_bass_kernel_spmd(
        nc, in_maps, core_ids=list(range(N_CORES)), trace=True
    )
    print("trace profile_json:", res.profile_json)
    _CACHE["last_res"] = res
    return res.exec_time_ns


if __name__ == "__main__":
    rng = np.random.default_rng(0)
    fake = {
        "x": rng.standard_normal((16, 64, 64, 64), np.float32),
        "conv1_w": rng.standard_normal((64, 64, 3, 3), np.float32) * 0.05,
        "conv1_b": rng.standard_normal((64,), np.float32) * 0.05,
        "bn1_gamma": rng.uniform(0.5, 1.5, (64,)).astype(np.float32),
        "bn1_beta": rng.standard_normal((64,), np.float32) * 0.05,
        "bn1_mean": rng.standard_normal((64,), np.float32) * 0.05,
        "bn1_var": rng.uniform(0.5, 1.5, (64,)).astype(np.float32),
        "conv2_w": rng.standard_normal((96, 64, 3, 3), np.float32) * 0.05,
        "conv2_b": rng.standard_normal((96,), np.float32) * 0.05,
        "bn2_gamma": rng.uniform(0.5, 1.5, (96,)).astype(np.float32),
        "bn2_beta": rng.standard_normal((96,), np.float32) * 0.05,
        "bn2_mean": rng.standard_normal((96,), np.float32) * 0.05,
        "bn2_var": rng.uniform(0.5, 1.5, (96,)).astype(np.float32),
        "bases": rng.standard_normal((6, 9), np.float32),
        "coef": rng.standard_normal((128, 1024), np.float32) * 0.02,
    }
    o = kernel(**fake)
    print("out", o.shape, o.dtype)
